# revision 60
# baseline (speedup 1.0000x reference)
"""Trainium2 Bass kernel for nn_CNN_Comp_29240137351522 (dense_cnn), v2.

Math:  y = |IFFT_N( FFT_N(x)^2 * C )|,  C = FFT_N(w0)^2 * FFT_N(wl) / N
with N = 2304 (= 128*18).  2304 >= 2303 covers the autoconv h*h exactly, and
the final circular conv aliases y[n+2304] only onto n < 255, which the center
crop [255:2303) discards, so the cropped result is exact.

Device decomposition per core (data-parallel over batch, S = 512 samples):
  n = n2*128 + n1 (n2 in [0,18), x nonzero for n2 < 8),  k = 18*k1 + k2
  F1 (contract n2, block-diag over j = n1 mod 16, twiddle folded, bf16)
  pivot-C (DMA)   -> Abig[n1, (k2, plane, s)]
  F3 (contract n1, shared W128, bf16) -> X[k1, (k2, s)] in PSUM
  square (ACT dual-bank Square + DVE ops) -> Zr, P2 = 2*Xr*Xi (bf16)
  I1 (contract k1, G = C-row-scaled inverse DFT built on device, bf16)
  pivot-D (DMA)   -> u2[(j, k2), (plane, s)]
  I2 (contract k2, block-diag over j, bf16) + |.|^2 + sqrt -> yraw (bf16)
Host does data movement only: batch shard, x permutation into the F1-ready
layout, packing of weight vectors, and the output unscramble.
"""

import numpy as np
import ml_dtypes

import concourse.bass as bass
import concourse.bacc as bacc
import concourse.mybir as mybir
from concourse.tile import TileContext
from concourse.bass_utils import run_bass_kernel_spmd

# ---------------- static problem config ----------------
B, NX = 4096, 1024
K0, KL = 129, 257
N = 2304
N1, N2 = 128, 18
NCORES = 8
S = B // NCORES              # 512 samples per core, single chunk
CROP0 = 255
CLASS_NUM = 2048
K2SPLIT = ((0, 8), (8, 16), (16, 18))     # F1 column splits (k2-major)
JBLK = ((0, 7), (7, 14), (14, 16))        # I2 j-blocks per g
F1COLS = 288                               # 18*16 cols per g
I2COLS = 272                               # 16*17 cols per g
YROWS = 119                                # max I2 out rows (7*17)

f32 = mybir.dt.float32
f32r = mybir.dt.float32r
bf16 = mybir.dt.bfloat16
AO = mybir.AluOpType
AF = mybir.ActivationFunctionType

BF = ml_dtypes.bfloat16


def _w(num, den):
    return np.exp(-2j * np.pi * np.asarray(num, np.float64) / den)


# ---------------- host-side constant arrays ----------------
def _build_consts():
    c = {}
    n1g = np.arange(N1)
    k1g = np.arange(N1)
    k2g = np.arange(N2)

    # F1 lhsT [128, 8*288]: row p = 8j + n2 ; col g*288 + sbase + k2sub*16 + j
    # value W18^{n2 k2} * W2304^{(16g+j) k2}
    f1 = np.zeros((128, 8 * F1COLS), np.complex128)
    for g in range(8):
        for (k2lo, k2hi), sbase in zip(K2SPLIT, (0, 128, 256)):
            nk = k2hi - k2lo
            for k2 in range(k2lo, k2hi):
                for j in range(16):
                    n1 = 16 * g + j
                    col = g * F1COLS + sbase + j * nk + (k2 - k2lo)
                    vals = _w(np.arange(8) * k2, N2) * _w(n1 * k2, N)
                    f1[8 * j : 8 * j + 8, col] = vals
    c["cf1"] = np.concatenate(
        [f1.real, f1.imag, -f1.imag], axis=1).astype(BF)   # [128, 3*2304]

    # F3 lhsT (shared): W128[n1,k1], bf16 + f32 copy for the weight-DFT mms
    w3 = _w(np.outer(n1g, k1g), N1)
    w3cat = np.concatenate([w3.real, w3.imag, -w3.imag], axis=1)
    c["cw3"] = w3cat.astype(BF)                            # [128, 384]
    c["cw3f"] = w3cat.astype(np.float32)                   # [128, 384]

    # inverse-DFT base tiled over k2, divided by N (folds the 1/N of C):
    # cwiB[:, v*2304 + k2*128 + p] = {Re,Im}(W128^{-k1 p}) / N
    wi = _w(-np.outer(k1g, n1g), N1) / N
    blk = np.concatenate([np.tile(wi.real, (1, N2)), np.tile(wi.imag, (1, N2))], axis=1)
    c["cwiB"] = blk.astype(BF)                             # [128, 2*2304]

    # I2 lhsT [128, 8*272]: per (g, jb): rows p = j'*18 + k2, col g*272 + base
    # + j'*17 + (q-1); value W18^{-q k2} * W2304^{-(16g+j0+j') k2}, q in [1,18)
    i2 = np.zeros((128, 8 * I2COLS), np.complex128)
    qg = np.arange(1, 18)
    for g in range(8):
        base = 0
        for (j0, j1) in JBLK:
            for jp in range(j1 - j0):
                n1 = 16 * g + j0 + jp
                blkv = _w(-np.outer(k2g, qg), N2) * _w(-n1 * k2g, N)[:, None]
                rows = slice(jp * 18, jp * 18 + 18)
                cols = slice(g * I2COLS + base + jp * 17, g * I2COLS + base + (jp + 1) * 17)
                i2[rows, cols] = blkv
            base += (j1 - j0) * 17
    c["ci2"] = np.concatenate(
        [i2.real, i2.imag, -i2.imag], axis=1).astype(BF)   # [128, 3*2176]

    # weight-DFT rhs constants (f32), packed into one [128, 272] tensor:
    # cols 0:18 ct1r | 18:36 ct1i | 36:54 ct2r | 54:72 ct2i
    # row0 cols 72:90 te1r | 90:108 te1i | 108:126 te2r | 126:144 te2i
    # row0 cols 144:272 ones (128)
    nh = np.arange(128)
    sm = np.zeros((128, 272), np.float32)
    t1 = _w(np.outer(nh, k2g), N)
    sm[:, 0:18] = t1.real
    sm[:, 18:36] = t1.imag
    t2 = _w(np.outer(nh, k2g), N) * _w(k2g, N2)[None, :]
    sm[:, 36:54] = t2.real
    sm[:, 54:72] = t2.imag
    te1 = _w(k2g, N2)
    sm[0, 72:90] = te1.real
    sm[0, 90:108] = te1.imag
    te2 = _w(k2g, 9)
    sm[0, 108:126] = te2.real
    sm[0, 126:144] = te2.imag
    sm[0, 144:272] = 1.0
    c["csm"] = sm

    return c


CONSTS = _build_consts()


# ---------------- bass kernel builder ----------------
def build_nc():
    nc = bacc.Bacc("TRN2", target_bir_lowering=False, debug=False, num_devices=NCORES)

    d = {}
    d["xt"] = nc.dram_tensor("xt", [128, 2 * 4096], bf16, kind="ExternalInput")
    d["wpack"] = nc.dram_tensor("wpack", [128, 10], f32, kind="ExternalInput")
    cdt = {"cw3f": f32, "csm": f32}
    for nm, arr in CONSTS.items():
        d[nm] = nc.dram_tensor(nm, list(arr.shape), cdt.get(nm, bf16), kind="ExternalInput")
    yraw = nc.dram_tensor("yraw", [YROWS, 8 * 1536], bf16, kind="ExternalOutput")

    with TileContext(nc) as tc:
        with (
            tc.tile_pool(name="cp", bufs=1) as cp,          # persistent consts
            tc.tile_pool(name="bp", bufs=1) as bp,          # Abig / Ubig / G
            tc.tile_pool(name="sp", bufs=3) as sp,          # rotating stage tiles
            tc.tile_pool(name="gp", bufs=2) as gp,          # G-build temporaries
            tc.tile_pool(name="stp", bufs=4) as stp,        # pivot-C staging
            tc.tile_pool(name="xp2", bufs=2) as xp2,        # xi/y copies
            tc.tile_pool(name="up", bufs=3) as up,          # u2 tiles
            tc.tile_pool(name="yp", bufs=4) as yp,          # yy tiles
            tc.tile_pool(name="zp", bufs=4) as zp,          # z tiles
            tc.tile_pool(name="tp", bufs=1) as tp,          # small f32 tmps
            tc.tile_pool(name="psa", bufs=2, space="PSUM") as psa,
        ):
            # ---- const + input DMAs (sync engine; ordered by need) ----
            # xt free layout: g*1024 + plane*512 + s  (per-g slices contiguous)
            wpk = cp.tile([128, 10], f32, tag="wpack")
            nc.sync.dma_start(out=wpk[:], in_=d["wpack"][:, :])
            csm = cp.tile([128, 272], f32, tag="csm")
            nc.sync.dma_start(out=csm[:], in_=d["csm"][:, :])
            cw3f = cp.tile([128, 384], f32, tag="cw3f")
            nc.sync.dma_start(out=cw3f[:], in_=d["cw3f"][:, :])
            cf1 = cp.tile([128, 3 * 2304], bf16, tag="cf1")
            for v in (0, 2, 1):
                vs = slice(v * 2304, (v + 1) * 2304)
                nc.sync.dma_start(out=cf1[:, vs], in_=d["cf1"][:, vs])
            xt = cp.tile([128, 8192], bf16, tag="xt")
            for g in range(8):
                gs = slice(g * 1024, (g + 1) * 1024)
                nc.sync.dma_start(out=xt[:, gs], in_=d["xt"][:, gs])
            cw3 = cp.tile([128, 384], bf16, tag="cw3")
            nc.sync.dma_start(out=cw3[:], in_=d["cw3"][:, :])
            cwiB = cp.tile([128, 2 * 2304], bf16, tag="cwiB")
            nc.sync.dma_start(out=cwiB[:], in_=d["cwiB"][:, :])
            ci2 = cp.tile([128, 3 * 2176], bf16, tag="ci2")

            # ---- weight DFT -> C (without 1/N; folded into cwiB) ----
            # rhs builds [rows, 18] f32  (complex products via DVE small ops)
            def cplx_rhs(rows, tr, ti, cr, ci, outr, outi):
                # (cr + i ci) * (tr + i ti); cr/ci are [rows,1] scalar APs
                t = tp.tile([128, 18], f32, tag="wtmp")
                nc.vector.tensor_scalar(t[:rows, :], ti, ci, None, AO.mult)
                nc.vector.scalar_tensor_tensor(outr, tr, cr, t[:rows, :], AO.mult, AO.subtract)
                t2 = tp.tile([128, 18], f32, tag="wtmp2")
                nc.vector.tensor_scalar(t2[:rows, :], tr, ci, None, AO.mult)
                nc.vector.scalar_tensor_tensor(outi, ti, cr, t2[:rows, :], AO.mult, AO.add)

            rhs0 = tp.tile([128, 36], f32, tag="rhs0")
            cplx_rhs(128, csm[:, 0:18], csm[:, 18:36], wpk[:, 0:1], wpk[:, 1:2],
                     rhs0[:, 0:18], rhs0[:, 18:36])
            tl0 = tp.tile([1, 36], f32, tag="tl0")
            cplx_rhs(1, csm[0:1, 72:90], csm[0:1, 90:108], wpk[0:1, 6:7], wpk[0:1, 7:8],
                     tl0[:, 0:18], tl0[:, 18:36])
            rhs1 = tp.tile([128, 36], f32, tag="rhs1")
            cplx_rhs(128, csm[:, 0:18], csm[:, 18:36], wpk[:, 2:3], wpk[:, 3:4],
                     rhs1[:, 0:18], rhs1[:, 18:36])
            rhs2 = tp.tile([128, 36], f32, tag="rhs2")
            cplx_rhs(128, csm[:, 36:54], csm[:, 54:72], wpk[:, 4:5], wpk[:, 5:6],
                     rhs2[:, 0:18], rhs2[:, 18:36])
            tl2 = tp.tile([1, 36], f32, tag="tl2")
            cplx_rhs(1, csm[0:1, 108:126], csm[0:1, 126:144], wpk[0:1, 8:9], wpk[0:1, 9:10],
                     tl2[:, 0:18], tl2[:, 18:36])

            w3fr = cw3f[:, 0:128]
            w3fi = cw3f[:, 128:256]
            w3fn = cw3f[:, 256:384]
            onesf = csm[0:1, 144:272]

            w0ps = psa.tile([128, 36], f32, tag="pX")
            nc.tensor.matmul(w0ps[:, 0:18], w3fr, rhs0[:, 0:18], start=True, stop=False)
            nc.tensor.matmul(w0ps[:, 0:18], w3fn, rhs0[:, 18:36], start=False, stop=False)
            nc.tensor.matmul(w0ps[:, 0:18], onesf, tl0[:, 0:18], start=False, stop=True)
            nc.tensor.matmul(w0ps[:, 18:36], w3fi, rhs0[:, 0:18], start=True, stop=False)
            nc.tensor.matmul(w0ps[:, 18:36], w3fr, rhs0[:, 18:36], start=False, stop=False)
            nc.tensor.matmul(w0ps[:, 18:36], onesf, tl0[:, 18:36], start=False, stop=True)
            wlps = psa.tile([128, 36], f32, tag="pU")
            nc.tensor.matmul(wlps[:, 0:18], w3fr, rhs1[:, 0:18], start=True, stop=False)
            nc.tensor.matmul(wlps[:, 0:18], w3fn, rhs1[:, 18:36], start=False, stop=False)
            nc.tensor.matmul(wlps[:, 0:18], w3fr, rhs2[:, 0:18], start=False, stop=False)
            nc.tensor.matmul(wlps[:, 0:18], w3fn, rhs2[:, 18:36], start=False, stop=False)
            nc.tensor.matmul(wlps[:, 0:18], onesf, tl2[:, 0:18], start=False, stop=True)
            nc.tensor.matmul(wlps[:, 18:36], w3fi, rhs1[:, 0:18], start=True, stop=False)
            nc.tensor.matmul(wlps[:, 18:36], w3fr, rhs1[:, 18:36], start=False, stop=False)
            nc.tensor.matmul(wlps[:, 18:36], w3fi, rhs2[:, 0:18], start=False, stop=False)
            nc.tensor.matmul(wlps[:, 18:36], w3fr, rhs2[:, 18:36], start=False, stop=False)
            nc.tensor.matmul(wlps[:, 18:36], onesf, tl2[:, 18:36], start=False, stop=True)

            w0sb = tp.tile([128, 36], f32, tag="w0sb")
            nc.scalar.activation(w0sb[:], w0ps[:], AF.Copy)
            wlsb = tp.tile([128, 36], f32, tag="wlsb")
            nc.scalar.activation(wlsb[:], wlps[:], AF.Copy)

            # C*N = W0^2 * WL  (f32, [128, 18] each)
            ca = tp.tile([128, 18], f32, tag="ca")
            cb = tp.tile([128, 18], f32, tag="cb")
            cm1 = tp.tile([128, 18], f32, tag="cm1")
            cm2 = tp.tile([128, 18], f32, tag="cm2")
            nc.vector.tensor_mul(cm1[:], w0sb[:, 0:18], w0sb[:, 0:18])
            nc.vector.tensor_mul(cm2[:], w0sb[:, 18:36], w0sb[:, 18:36])
            nc.vector.tensor_sub(ca[:], cm1[:], cm2[:])
            nc.vector.scalar_tensor_tensor(cb[:], w0sb[:, 0:18], 2.0, w0sb[:, 18:36],
                                           AO.mult, AO.mult)
            crn = tp.tile([128, 18], f32, tag="crn")
            cin = tp.tile([128, 18], f32, tag="cin")
            nc.vector.tensor_mul(cm1[:], ca[:], wlsb[:, 0:18])
            nc.vector.tensor_mul(cm2[:], cb[:], wlsb[:, 18:36])
            nc.vector.tensor_sub(crn[:], cm1[:], cm2[:])
            nc.vector.tensor_mul(cm1[:], ca[:], wlsb[:, 18:36])
            nc.vector.tensor_mul(cm2[:], cb[:], wlsb[:, 0:18])
            nc.vector.tensor_add(cin[:], cm1[:], cm2[:])

            # ---- G build: G = (wi/N) * C*N, [128, 2304] bf16 x3 ----
            # per-k2 pieces on DVE or Pool (SBUF-only, hw-legal); emitted
            # interleaved into phase A so they don't block phase-A evictions
            Gr = bp.tile([128, 2304], bf16, tag="Gr")
            Gi = bp.tile([128, 2304], bf16, tag="Gi")
            Gn = bp.tile([128, 2304], bf16, tag="Gn")
            wbr = cwiB[:, 0:2304]
            wbi = cwiB[:, 2304:4608]

            def g_piece(k2, eng):
                v = nc.vector if eng == "dve" else nc.gpsimd
                ksl = slice(k2 * 128, (k2 + 1) * 128)
                crc = crn[:, k2:k2 + 1]
                cic = cin[:, k2:k2 + 1]
                gA = gp.tile([128, 128], bf16, tag="gA")
                v.tensor_scalar(gA[:], wbi[:, ksl], cic, None, AO.mult)
                v.scalar_tensor_tensor(Gr[:, ksl], wbr[:, ksl], crc, gA[:],
                                       AO.mult, AO.subtract)
                gB = gp.tile([128, 128], bf16, tag="gB")
                v.tensor_scalar(gB[:], wbr[:, ksl], cic, None, AO.mult)
                v.scalar_tensor_tensor(Gi[:, ksl], wbi[:, ksl], crc, gB[:],
                                       AO.mult, AO.add)
                v.tensor_scalar(Gn[:, ksl], Gi[:, ksl], -1.0, None, AO.mult)

            cf1r = cf1[:, 0:2304]
            cf1i = cf1[:, 2304:4608]
            cf1n = cf1[:, 4608:6912]
            w3r = cw3[:, 0:128]
            w3i = cw3[:, 128:256]
            w3n = cw3[:, 256:384]

            Abig = bp.tile([128, 18432], bf16, tag="Abig")
            Ubig = bp.tile([128, 18432], bf16, tag="Ubig")

            def ev_op(engine, dst, src):
                # pool/gpsimd cannot access PSUM on TRN2 hardware
                if engine == "act":
                    nc.scalar.activation(dst, src, AF.Copy)
                else:
                    nc.vector.tensor_copy(dst, src)

            # ---- Phase A: F1 + pivot-C (evict/DMA lagged one iteration) ----
            # DVE runs the G-build and Pool/ACT the crE expansion early; keep
            # phase-A evictions off DVE entirely (in-order queues would stall
            # F1 behind the G-build) and give Pool only the tail.
            EVA = ["act"] * 24
            # G pieces: (emit-after-iteration, k2, engine)
            GSCHED = {}
            _gk2 = 0
            for _it in range(2, 24):
                if _it % 4 != 1:
                    GSCHED[_it] = (_gk2, "dve")
                    _gk2 += 1
            # remaining k2 values flushed after phase A
            GREST = list(range(_gk2, N2))
            fa_pend = []  # (psum, rows, g, k2lo, nk)

            def fa_flush(idx):
                ab, rows, g, k2lo, nk = fa_pend[idx]
                stg = stp.tile([128, 1024], bf16, tag="stg")
                ev_op(EVA[idx], stg[:rows, :], ab[:rows, :])
                nc.sync.dma_start(
                    out=bass.AP(Abig.tensor,
                                Abig[:].offset + (16 * g) * 18432 + k2lo * 1024,
                                [[18432, 16], [1024, nk], [1, 1024]]),
                    in_=bass.AP(stg.tensor, stg[:].offset,
                                [[1024, 16 * nk], [1, 1024]]),
                )

            it = 0
            for si, (k2lo, k2hi) in enumerate(K2SPLIT):
                nk = k2hi - k2lo
                rows = nk * 16
                sbase = si * 128
                for g in range(8):
                    csl = slice(g * F1COLS + sbase, g * F1COLS + sbase + rows)
                    xr = xt[:, g * 1024 : g * 1024 + 512]
                    xi = xt[:, g * 1024 + 512 : (g + 1) * 1024]
                    ab = psa.tile([128, 1024], f32, tag="pX" if it % 2 == 0 else "pU")
                    nc.tensor.matmul(ab[:rows, 0:512], cf1r[:, csl], xr, start=True, stop=False)
                    nc.tensor.matmul(ab[:rows, 0:512], cf1n[:, csl], xi, start=False, stop=True)
                    nc.tensor.matmul(ab[:rows, 512:1024], cf1i[:, csl], xr, start=True, stop=False)
                    nc.tensor.matmul(ab[:rows, 512:1024], cf1r[:, csl], xi, start=False, stop=True)
                    fa_pend.append((ab, rows, g, k2lo, nk))
                    if it >= 1:
                        fa_flush(it - 1)
                    if it in GSCHED:
                        g_piece(*GSCHED[it])
                    it += 1
            fa_flush(it - 1)
            for _k2 in GREST:
                g_piece(_k2, "dve")
            # late const loads: needed only in phase C
            for v in range(3):
                vs = slice(v * 2176, (v + 1) * 2176)
                nc.sync.dma_start(out=ci2[:, vs], in_=d["ci2"][:, vs])

            # ---- Phase B: F3 + square + I1, software-pipelined by k2 ----
            # per-iteration emit: F3(k2), I1(k2-1), sq(k2), P2(k2), Zr(k2), ev(k2-1)
            XCE = ["act", "dve", "dve", "act", "dve", "act", "dve", "dve", "act",
                   "dve", "act", "dve", "dve", "act", "dve", "act", "dve", "act"]
            ZRE = ["pool", "pool", "dve", "pool", "pool", "dve", "pool", "pool",
                   "dve", "pool", "pool", "dve", "pool", "pool", "dve", "pool",
                   "pool", "dve"]
            EVB = ["act", "dve", "dve", "act", "act", "dve", "dve", "act", "dve",
                   "act", "dve", "dve", "act", "act", "dve", "dve", "act", "act"]
            zt = [None] * N2
            upst = [None] * N2

            def b_f3(k2):
                ar = Abig[:, k2 * 1024 : k2 * 1024 + 512]
                ai = Abig[:, k2 * 1024 + 512 : (k2 + 1) * 1024]
                xps = psa.tile([128, 1024], f32, tag="pX")
                nc.tensor.matmul(xps[:, 0:512], w3r, ar, start=True, stop=False)
                nc.tensor.matmul(xps[:, 0:512], w3n, ai, start=False, stop=True)
                nc.tensor.matmul(xps[:, 512:1024], w3i, ar, start=True, stop=False)
                nc.tensor.matmul(xps[:, 512:1024], w3r, ai, start=False, stop=True)
                return xps

            def b_i1(k2):
                z = zt[k2]
                gsl = slice(k2 * 128, (k2 + 1) * 128)
                ups = psa.tile([128, 1024], f32, tag="pU")
                nc.tensor.matmul(ups[:, 0:512], Gr[:, gsl], z[:, 0:512], start=True, stop=False)
                nc.tensor.matmul(ups[:, 0:512], Gn[:, gsl], z[:, 512:1024], start=False, stop=True)
                nc.tensor.matmul(ups[:, 512:1024], Gi[:, gsl], z[:, 0:512], start=True, stop=False)
                nc.tensor.matmul(ups[:, 512:1024], Gr[:, gsl], z[:, 512:1024], start=False, stop=True)
                upst[k2] = ups

            sqt = [None] * N2

            def b_sqp2(k2, xps):
                # hw: only one non-scalar input may be in PSUM; squares go
                # through ACT (single input), the cross product via an SBUF
                # copy of Xi
                sq = sp.tile([128, 1024], bf16, tag="sq")
                nc.scalar.activation(sq[:], xps[:], AF.Square)
                xic = xp2.tile([128, 512], bf16, tag="xic")
                if XCE[k2] == "act":
                    nc.scalar.activation(xic[:], xps[:, 512:1024], AF.Copy)
                else:
                    nc.vector.tensor_copy(xic[:], xps[:, 512:1024])
                sqt[k2] = sq
                z = zp.tile([128, 1024], bf16, tag="z")
                nc.vector.scalar_tensor_tensor(z[:, 512:1024], xps[:, 0:512], 2.0,
                                               xic[:], AO.mult, AO.mult)
                zt[k2] = z

            def b_zr(k2):
                v = nc.gpsimd if ZRE[k2] == "pool" else nc.vector
                v.tensor_tensor(zt[k2][:, 0:512], sqt[k2][:, 0:512], sqt[k2][:, 512:1024],
                                AO.subtract)

            def b_ev(k2):
                ev_op(EVB[k2], Ubig[:, k2 * 1024 : (k2 + 1) * 1024], upst[k2][:])

            for k2 in range(N2):
                xps = b_f3(k2)
                if k2 >= 3:
                    b_i1(k2 - 3)
                b_sqp2(k2, xps)
                if k2 >= 1:
                    b_zr(k2 - 1)
                if k2 >= 4:
                    b_ev(k2 - 4)
            b_zr(N2 - 1)
            for k2 in (N2 - 3, N2 - 2, N2 - 1):
                b_i1(k2)
            for k2 in (N2 - 4, N2 - 3, N2 - 2, N2 - 1):
                b_ev(k2)

            # ---- Phase C: pivot-D + I2 + |.| + store, pipelined by (g, blk) ----
            ci2r = ci2[:, 0:2176]
            ci2i = ci2[:, 2176:4352]
            ci2n = ci2[:, 4352:6528]
            POSTE = ["act", "act", "dve", "act", "dve", "dve"] * 4
            ADDE = ["pool", "pool", "dve"] * 8

            def c_pivd(g):
                tiles = []
                for bi, (j0, j1) in enumerate(JBLK):
                    cnt = j1 - j0
                    u2 = up.tile([128, 1024], bf16, tag=f"u2{bi}")
                    nc.sync.dma_start(
                        out=bass.AP(u2.tensor, u2[:].offset,
                                    [[1024, cnt * 18], [1, 1024]]),
                        in_=bass.AP(Ubig.tensor,
                                    Ubig[:].offset + (16 * g + j0) * 18432,
                                    [[18432, cnt], [1024, 18], [1, 1024]]),
                    )
                    tiles.append(u2)
                return tiles

            CB = [(0, 0, 119), (1, 119, 119), (2, 238, 34)]  # (blk, colbase, rows)

            def c_mm(g, bi, u2):
                j0, j1 = JBLK[bi]
                cnt = j1 - j0
                rows = CB[bi][2]
                parts = cnt * 18
                csl = slice(g * I2COLS + CB[bi][1], g * I2COLS + CB[bi][1] + rows)
                yps = psa.tile([128, 1024], f32, tag="pX" if (g + bi) % 2 == 0 else "pU")
                nc.tensor.matmul(yps[:rows, 0:512], ci2r[:parts, csl], u2[:parts, 0:512],
                                 start=True, stop=False)
                nc.tensor.matmul(yps[:rows, 0:512], ci2n[:parts, csl], u2[:parts, 512:1024],
                                 start=False, stop=True)
                nc.tensor.matmul(yps[:rows, 512:1024], ci2i[:parts, csl], u2[:parts, 0:512],
                                 start=True, stop=False)
                nc.tensor.matmul(yps[:rows, 512:1024], ci2r[:parts, csl], u2[:parts, 512:1024],
                                 start=False, stop=True)
                return yps

            def c_post(g, bi, yps, yy):
                rows = CB[bi][2]
                m = sp.tile([128, 1024], bf16, tag="m")
                pe = POSTE[(g * 3 + bi) % len(POSTE)]
                if pe == "act":
                    nc.scalar.activation(m[:rows, :], yps[:rows, :], AF.Square)
                else:
                    mc = xp2.tile([128, 1024], bf16, tag="mc")
                    nc.vector.tensor_copy(mc[:rows, :], yps[:rows, :])
                    nc.vector.tensor_mul(m[:rows, :], mc[:rows, :], mc[:rows, :])
                va = nc.gpsimd if ADDE[(g * 3 + bi) % len(ADDE)] == "pool" else nc.vector
                va.tensor_tensor(yy[:rows, bi * 512 : (bi + 1) * 512],
                                 m[:rows, 0:512], m[:rows, 512:1024], AO.add)

            # steady state per g: pivD(g+2), mm(g,*) with post lag 1 blk,
            # sqrt(g-1) right after post(g-1,2), out-DMA(g-2) last (so the
            # SP queue never parks on an unmet sqrt wait ahead of pivDs).
            u2_0 = c_pivd(0)
            u2_1 = c_pivd(1)
            u2_t = {0: u2_0, 1: u2_1}
            yy_t = {}
            pend = []  # (g, bi, yps)
            fin = []   # g values whose posts are all emitted
            for g in range(8):
                yyg = yp.tile([128, 1536], bf16, tag="yy")
                nc.gpsimd.memset(yyg[0:YROWS, 1024:1536], 0.0)
                yy_t[g] = yyg
                if g + 2 < 8:
                    u2_t[g + 2] = c_pivd(g + 2)
                for bi in range(3):
                    yps = c_mm(g, bi, u2_t[g][bi])
                    if len(pend) >= 2:
                        pg, pbi, pyps = pend.pop(0)
                        c_post(pg, pbi, pyps, yy_t[pg])
                        if pbi == 2:
                            nc.scalar.activation(yy_t[pg][:YROWS, :], yy_t[pg][:YROWS, :], AF.Sqrt)
                            fin.append(pg)
                    pend.append((g, bi, yps))
                if len(fin) >= 2:
                    og = fin.pop(0)
                    nc.sync.dma_start(
                        out=yraw[0:YROWS, og * 1536 : (og + 1) * 1536],
                        in_=yy_t[og][:YROWS, :],
                    )
            while pend:
                pg, pbi, pyps = pend.pop(0)
                c_post(pg, pbi, pyps, yy_t[pg])
                if pbi == 2:
                    nc.scalar.activation(yy_t[pg][:YROWS, :], yy_t[pg][:YROWS, :], AF.Sqrt)
                    fin.append(pg)
            for og in fin:
                nc.sync.dma_start(
                    out=yraw[0:YROWS, og * 1536 : (og + 1) * 1536],
                    in_=yy_t[og][:YROWS, :],
                )

    nc.compile()
    return nc


_NC_CACHE = None


# ---------------- host-side orchestration ----------------
def _host_x(x_real, x_imag):
    """[Bc, 1024] f32 -> xt [128, 8192] bf16: p = 8j+n2, free = g*1024+plane*512+s."""
    out = np.empty((NCORES, 128, 8, 2, 512), BF)
    for cid in range(NCORES):
        rows = slice(cid * S, (cid + 1) * S)
        for pi, arr in enumerate((x_real, x_imag)):
            a = arr[rows].reshape(S, 8, 8, 16)          # (s, n2, g, j)
            a = a.transpose(3, 1, 2, 0)                 # (j, n2, g, s)
            out[cid, :, :, pi, :] = a.reshape(128, 8, S).astype(BF)
    return out.reshape(NCORES, 128, 8192)


def _build_wpack(w0r, w0i, wlr, wli):
    wp = np.zeros((128, 10), np.float32)
    wp[:, 0] = w0r[0:128]
    wp[:, 1] = w0i[0:128]
    wp[:, 2] = wlr[0:128]
    wp[:, 3] = wli[0:128]
    wp[:, 4] = wlr[128:256]
    wp[:, 5] = wli[128:256]
    wp[0, 6] = w0r[128]
    wp[0, 7] = w0i[128]
    wp[0, 8] = wlr[256]
    wp[0, 9] = wli[256]
    return wp


def _out_maps():
    """(rows, cols_in_yraw_per_g, out_col) for valid outputs."""
    rr, cc, oo = [], [], []
    for bi, (j0, j1) in enumerate(JBLK):
        for jp in range(j1 - j0):
            for qi in range(17):
                q = qi + 1
                r = jp * 17 + qi
                for g in range(8):
                    n = q * 128 + 16 * g + j0 + jp
                    if CROP0 <= n < CROP0 + CLASS_NUM:
                        rr.append(r)
                        cc.append(g * 1536 + bi * 512)
                        oo.append(n - CROP0)
    return np.array(rr), np.array(cc), np.array(oo)


_OUT_R, _OUT_C, _OUT_O = _out_maps()


def kernel(**inputs):
    global _NC_CACHE
    x_real = np.ascontiguousarray(inputs["x_real"], dtype=np.float32)
    x_imag = np.ascontiguousarray(inputs["x_imag"], dtype=np.float32)
    w0r = np.ascontiguousarray(inputs["w0_real"], dtype=np.float32)
    w0i = np.ascontiguousarray(inputs["w0_imag"], dtype=np.float32)
    wlr = np.ascontiguousarray(inputs["wl_real"], dtype=np.float32)
    wli = np.ascontiguousarray(inputs["wl_imag"], dtype=np.float32)

    xts = _host_x(x_real, x_imag)
    wp = _build_wpack(w0r, w0i, wlr, wli)

    const_maps = {nm: np.ascontiguousarray(arr) for nm, arr in CONSTS.items()}
    in_maps = []
    for cid in range(NCORES):
        m = {"xt": np.ascontiguousarray(xts[cid]), "wpack": wp}
        m.update(const_maps)
        in_maps.append(m)

    if _NC_CACHE is None:
        _NC_CACHE = build_nc()
    res = run_bass_kernel_spmd(_NC_CACHE, in_maps, core_ids=list(range(NCORES)))

    out = np.empty((B, CLASS_NUM), np.float32)
    for cid in range(NCORES):
        yr = np.asarray(res.results[cid]["yraw"], dtype=np.float32)  # [119, 12288]
        # gather: out[s, oo] = yr[rr, cc + s]
        sub = yr[_OUT_R[:, None], _OUT_C[:, None] + np.arange(S)[None, :]]  # [nv, S]
        out[cid * S : (cid + 1) * S, _OUT_O] = sub.T
    return out


# revision 61
# speedup vs baseline: 1.0529x; 1.0529x over previous
"""Trainium2 Bass kernel for nn_CNN_Comp_29240137351522 (dense_cnn), v2.

Math:  y = |IFFT_N( FFT_N(x)^2 * C )|,  C = FFT_N(w0)^2 * FFT_N(wl) / N
with N = 2304 (= 128*18).  2304 >= 2303 covers the autoconv h*h exactly, and
the final circular conv aliases y[n+2304] only onto n < 255, which the center
crop [255:2303) discards, so the cropped result is exact.

Device decomposition per core (data-parallel over batch, S = 512 samples):
  n = n2*128 + n1 (n2 in [0,18), x nonzero for n2 < 8),  k = 18*k1 + k2
  F1 (contract n2, block-diag over j = n1 mod 16, twiddle folded, bf16)
  pivot-C (DMA)   -> Abig[n1, (k2, plane, s)]
  F3 (contract n1, shared W128, bf16) -> X[k1, (k2, s)] in PSUM
  square (ACT dual-bank Square + DVE ops) -> Zr, P2 = 2*Xr*Xi (bf16)
  I1 (contract k1, G = C-row-scaled inverse DFT built on device, bf16)
  pivot-D (DMA)   -> u2[(j, k2), (plane, s)]
  I2 (contract k2, block-diag over j, bf16) + |.|^2 + sqrt -> yraw (bf16)
Host does data movement only: batch shard, x permutation into the F1-ready
layout, packing of weight vectors, and the output unscramble.
"""

import numpy as np
import ml_dtypes

import concourse.bass as bass
import concourse.bacc as bacc
import concourse.mybir as mybir
from concourse.tile import TileContext
from concourse.bass_utils import run_bass_kernel_spmd

# ---------------- static problem config ----------------
B, NX = 4096, 1024
K0, KL = 129, 257
N = 2304
N1, N2 = 128, 18
NCORES = 8
S = B // NCORES              # 512 samples per core, single chunk
CROP0 = 255
CLASS_NUM = 2048
K2SPLIT = ((0, 8), (8, 16), (16, 18))     # F1 column splits (k2-major)
JBLK = ((0, 7), (7, 14), (14, 16))        # I2 j-blocks per g
F1COLS = 288                               # 18*16 cols per g
I2COLS = 272                               # 16*17 cols per g
YROWS = 119                                # max I2 out rows (7*17)

f32 = mybir.dt.float32
f32r = mybir.dt.float32r
bf16 = mybir.dt.bfloat16
AO = mybir.AluOpType
AF = mybir.ActivationFunctionType

BF = ml_dtypes.bfloat16


def _w(num, den):
    return np.exp(-2j * np.pi * np.asarray(num, np.float64) / den)


# ---------------- host-side constant arrays ----------------
def _build_consts():
    c = {}
    n1g = np.arange(N1)
    k1g = np.arange(N1)
    k2g = np.arange(N2)

    # F1 lhsT [128, 8*288]: row p = 8j + n2 ; col g*288 + sbase + k2sub*16 + j
    # value W18^{n2 k2} * W2304^{(16g+j) k2}
    f1 = np.zeros((128, 8 * F1COLS), np.complex128)
    for g in range(8):
        for (k2lo, k2hi), sbase in zip(K2SPLIT, (0, 128, 256)):
            nk = k2hi - k2lo
            for k2 in range(k2lo, k2hi):
                for j in range(16):
                    n1 = 16 * g + j
                    col = g * F1COLS + sbase + j * nk + (k2 - k2lo)
                    vals = _w(np.arange(8) * k2, N2) * _w(n1 * k2, N)
                    f1[8 * j : 8 * j + 8, col] = vals
    c["cf1"] = np.concatenate(
        [f1.real, f1.imag, -f1.imag], axis=1).astype(BF)   # [128, 3*2304]

    # F3 lhsT (shared): W128[n1,k1], bf16 + f32 copy for the weight-DFT mms
    w3 = _w(np.outer(n1g, k1g), N1)
    w3cat = np.concatenate([w3.real, w3.imag, -w3.imag], axis=1)
    c["cw3"] = w3cat.astype(BF)                            # [128, 384]
    c["cw3f"] = w3cat.astype(np.float32)                   # [128, 384]

    # inverse-DFT base tiled over k2, divided by N (folds the 1/N of C):
    # cwiB[:, v*2304 + k2*128 + p] = {Re,Im}(W128^{-k1 p}) / N
    wi = _w(-np.outer(k1g, n1g), N1) / N
    blk = np.concatenate([np.tile(wi.real, (1, N2)), np.tile(wi.imag, (1, N2))], axis=1)
    c["cwiB"] = blk.astype(BF)                             # [128, 2*2304]

    # I2 lhsT [128, 8*272]: per (g, jb): rows p = j'*18 + k2, col g*272 + base
    # + j'*17 + (q-1); value W18^{-q k2} * W2304^{-(16g+j0+j') k2}, q in [1,18)
    i2 = np.zeros((128, 8 * I2COLS), np.complex128)
    qg = np.arange(1, 18)
    for g in range(8):
        base = 0
        for (j0, j1) in JBLK:
            for jp in range(j1 - j0):
                n1 = 16 * g + j0 + jp
                blkv = _w(-np.outer(k2g, qg), N2) * _w(-n1 * k2g, N)[:, None]
                rows = slice(jp * 18, jp * 18 + 18)
                cols = slice(g * I2COLS + base + jp * 17, g * I2COLS + base + (jp + 1) * 17)
                i2[rows, cols] = blkv
            base += (j1 - j0) * 17
    c["ci2"] = np.concatenate(
        [i2.real, i2.imag, -i2.imag], axis=1).astype(BF)   # [128, 3*2176]

    # weight-DFT rhs constants (f32), packed into one [128, 272] tensor:
    # cols 0:18 ct1r | 18:36 ct1i | 36:54 ct2r | 54:72 ct2i
    # row0 cols 72:90 te1r | 90:108 te1i | 108:126 te2r | 126:144 te2i
    # row0 cols 144:272 ones (128)
    nh = np.arange(128)
    sm = np.zeros((128, 272), np.float32)
    t1 = _w(np.outer(nh, k2g), N)
    sm[:, 0:18] = t1.real
    sm[:, 18:36] = t1.imag
    t2 = _w(np.outer(nh, k2g), N) * _w(k2g, N2)[None, :]
    sm[:, 36:54] = t2.real
    sm[:, 54:72] = t2.imag
    te1 = _w(k2g, N2)
    sm[0, 72:90] = te1.real
    sm[0, 90:108] = te1.imag
    te2 = _w(k2g, 9)
    sm[0, 108:126] = te2.real
    sm[0, 126:144] = te2.imag
    sm[0, 144:272] = 1.0
    c["csm"] = sm

    return c


CONSTS = _build_consts()


# ---------------- bass kernel builder ----------------
def build_nc():
    nc = bacc.Bacc("TRN2", target_bir_lowering=False, debug=False, num_devices=NCORES)

    d = {}
    d["xt"] = nc.dram_tensor("xt", [128, 2 * 4096], bf16, kind="ExternalInput")
    d["wpack"] = nc.dram_tensor("wpack", [128, 10], f32, kind="ExternalInput")
    cdt = {"cw3f": f32, "csm": f32}
    for nm, arr in CONSTS.items():
        d[nm] = nc.dram_tensor(nm, list(arr.shape), cdt.get(nm, bf16), kind="ExternalInput")
    yraw = nc.dram_tensor("yraw", [YROWS, 8 * 1536], bf16, kind="ExternalOutput")

    with TileContext(nc) as tc:
        with (
            tc.tile_pool(name="cp", bufs=1) as cp,          # persistent consts
            tc.tile_pool(name="bp", bufs=1) as bp,          # Abig / Ubig / G
            tc.tile_pool(name="sp", bufs=3) as sp,          # rotating stage tiles
            tc.tile_pool(name="gp", bufs=2) as gp,          # G-build temporaries
            tc.tile_pool(name="stp", bufs=4) as stp,        # pivot-C staging
            tc.tile_pool(name="xp2", bufs=2) as xp2,        # xi/y copies
            tc.tile_pool(name="up", bufs=3) as up,          # u2 tiles
            tc.tile_pool(name="yp", bufs=4) as yp,          # yy tiles
            tc.tile_pool(name="zp", bufs=4) as zp,          # z tiles
            tc.tile_pool(name="tp", bufs=1) as tp,          # small f32 tmps
            tc.tile_pool(name="psa", bufs=2, space="PSUM") as psa,
        ):
            # ---- const + input DMAs (sync engine; ordered by need) ----
            # xt free layout: g*1024 + plane*512 + s  (per-g slices contiguous)
            wpk = cp.tile([128, 10], f32, tag="wpack")
            nc.sync.dma_start(out=wpk[:], in_=d["wpack"][:, :])
            csm = cp.tile([128, 272], f32, tag="csm")
            nc.sync.dma_start(out=csm[:], in_=d["csm"][:, :])
            cw3f = cp.tile([128, 384], f32, tag="cw3f")
            nc.sync.dma_start(out=cw3f[:], in_=d["cw3f"][:, :])
            cf1 = cp.tile([128, 3 * 2304], bf16, tag="cf1")
            for v in (0, 2, 1):
                vs = slice(v * 2304, (v + 1) * 2304)
                nc.sync.dma_start(out=cf1[:, vs], in_=d["cf1"][:, vs])
            xt = cp.tile([128, 8192], bf16, tag="xt")
            for g in range(8):
                gs = slice(g * 1024, (g + 1) * 1024)
                nc.sync.dma_start(out=xt[:, gs], in_=d["xt"][:, gs])
            cw3 = cp.tile([128, 384], bf16, tag="cw3")
            nc.sync.dma_start(out=cw3[:], in_=d["cw3"][:, :])
            cwiB = cp.tile([128, 2 * 2304], bf16, tag="cwiB")
            nc.sync.dma_start(out=cwiB[:], in_=d["cwiB"][:, :])
            ci2 = cp.tile([128, 3 * 2176], bf16, tag="ci2")

            # ---- weight DFT -> C (without 1/N; folded into cwiB) ----
            # rhs builds [rows, 18] f32  (complex products via DVE small ops)
            def cplx_rhs(rows, tr, ti, cr, ci, outr, outi):
                # (cr + i ci) * (tr + i ti); cr/ci are [rows,1] scalar APs
                t = tp.tile([128, 18], f32, tag="wtmp")
                nc.vector.tensor_scalar(t[:rows, :], ti, ci, None, AO.mult)
                nc.vector.scalar_tensor_tensor(outr, tr, cr, t[:rows, :], AO.mult, AO.subtract)
                t2 = tp.tile([128, 18], f32, tag="wtmp2")
                nc.vector.tensor_scalar(t2[:rows, :], tr, ci, None, AO.mult)
                nc.vector.scalar_tensor_tensor(outi, ti, cr, t2[:rows, :], AO.mult, AO.add)

            rhs0 = tp.tile([128, 36], f32, tag="rhs0")
            cplx_rhs(128, csm[:, 0:18], csm[:, 18:36], wpk[:, 0:1], wpk[:, 1:2],
                     rhs0[:, 0:18], rhs0[:, 18:36])
            tl0 = tp.tile([1, 36], f32, tag="tl0")
            cplx_rhs(1, csm[0:1, 72:90], csm[0:1, 90:108], wpk[0:1, 6:7], wpk[0:1, 7:8],
                     tl0[:, 0:18], tl0[:, 18:36])
            rhs1 = tp.tile([128, 36], f32, tag="rhs1")
            cplx_rhs(128, csm[:, 0:18], csm[:, 18:36], wpk[:, 2:3], wpk[:, 3:4],
                     rhs1[:, 0:18], rhs1[:, 18:36])
            rhs2 = tp.tile([128, 36], f32, tag="rhs2")
            cplx_rhs(128, csm[:, 36:54], csm[:, 54:72], wpk[:, 4:5], wpk[:, 5:6],
                     rhs2[:, 0:18], rhs2[:, 18:36])
            tl2 = tp.tile([1, 36], f32, tag="tl2")
            cplx_rhs(1, csm[0:1, 108:126], csm[0:1, 126:144], wpk[0:1, 8:9], wpk[0:1, 9:10],
                     tl2[:, 0:18], tl2[:, 18:36])

            w3fr = cw3f[:, 0:128]
            w3fi = cw3f[:, 128:256]
            w3fn = cw3f[:, 256:384]
            onesf = csm[0:1, 144:272]

            w0ps = psa.tile([128, 36], f32, tag="pX")
            nc.tensor.matmul(w0ps[:, 0:18], w3fr, rhs0[:, 0:18], start=True, stop=False)
            nc.tensor.matmul(w0ps[:, 0:18], w3fn, rhs0[:, 18:36], start=False, stop=False)
            nc.tensor.matmul(w0ps[:, 0:18], onesf, tl0[:, 0:18], start=False, stop=True)
            nc.tensor.matmul(w0ps[:, 18:36], w3fi, rhs0[:, 0:18], start=True, stop=False)
            nc.tensor.matmul(w0ps[:, 18:36], w3fr, rhs0[:, 18:36], start=False, stop=False)
            nc.tensor.matmul(w0ps[:, 18:36], onesf, tl0[:, 18:36], start=False, stop=True)
            wlps = psa.tile([128, 36], f32, tag="pU")
            nc.tensor.matmul(wlps[:, 0:18], w3fr, rhs1[:, 0:18], start=True, stop=False)
            nc.tensor.matmul(wlps[:, 0:18], w3fn, rhs1[:, 18:36], start=False, stop=False)
            nc.tensor.matmul(wlps[:, 0:18], w3fr, rhs2[:, 0:18], start=False, stop=False)
            nc.tensor.matmul(wlps[:, 0:18], w3fn, rhs2[:, 18:36], start=False, stop=False)
            nc.tensor.matmul(wlps[:, 0:18], onesf, tl2[:, 0:18], start=False, stop=True)
            nc.tensor.matmul(wlps[:, 18:36], w3fi, rhs1[:, 0:18], start=True, stop=False)
            nc.tensor.matmul(wlps[:, 18:36], w3fr, rhs1[:, 18:36], start=False, stop=False)
            nc.tensor.matmul(wlps[:, 18:36], w3fi, rhs2[:, 0:18], start=False, stop=False)
            nc.tensor.matmul(wlps[:, 18:36], w3fr, rhs2[:, 18:36], start=False, stop=False)
            nc.tensor.matmul(wlps[:, 18:36], onesf, tl2[:, 18:36], start=False, stop=True)

            w0sb = tp.tile([128, 36], f32, tag="w0sb")
            nc.scalar.activation(w0sb[:], w0ps[:], AF.Copy)
            wlsb = tp.tile([128, 36], f32, tag="wlsb")
            nc.scalar.activation(wlsb[:], wlps[:], AF.Copy)

            # C*N = W0^2 * WL  (f32, [128, 18] each)
            ca = tp.tile([128, 18], f32, tag="ca")
            cb = tp.tile([128, 18], f32, tag="cb")
            cm1 = tp.tile([128, 18], f32, tag="cm1")
            cm2 = tp.tile([128, 18], f32, tag="cm2")
            nc.vector.tensor_mul(cm1[:], w0sb[:, 0:18], w0sb[:, 0:18])
            nc.vector.tensor_mul(cm2[:], w0sb[:, 18:36], w0sb[:, 18:36])
            nc.vector.tensor_sub(ca[:], cm1[:], cm2[:])
            nc.vector.scalar_tensor_tensor(cb[:], w0sb[:, 0:18], 2.0, w0sb[:, 18:36],
                                           AO.mult, AO.mult)
            crn = tp.tile([128, 18], f32, tag="crn")
            cin = tp.tile([128, 18], f32, tag="cin")
            nc.vector.tensor_mul(cm1[:], ca[:], wlsb[:, 0:18])
            nc.vector.tensor_mul(cm2[:], cb[:], wlsb[:, 18:36])
            nc.vector.tensor_sub(crn[:], cm1[:], cm2[:])
            nc.vector.tensor_mul(cm1[:], ca[:], wlsb[:, 18:36])
            nc.vector.tensor_mul(cm2[:], cb[:], wlsb[:, 0:18])
            nc.vector.tensor_add(cin[:], cm1[:], cm2[:])

            # ---- G build: G = (wi/N) * C*N, [128, 2304] bf16 x3 ----
            # per-k2 pieces on DVE or Pool (SBUF-only, hw-legal); emitted
            # interleaved into phase A so they don't block phase-A evictions
            Gr = bp.tile([128, 2304], bf16, tag="Gr")
            Gi = bp.tile([128, 2304], bf16, tag="Gi")
            Gn = bp.tile([128, 2304], bf16, tag="Gn")
            wbr = cwiB[:, 0:2304]
            wbi = cwiB[:, 2304:4608]

            def g_piece(k2, eng):
                v = nc.vector if eng == "dve" else nc.gpsimd
                ksl = slice(k2 * 128, (k2 + 1) * 128)
                crc = crn[:, k2:k2 + 1]
                cic = cin[:, k2:k2 + 1]
                gA = gp.tile([128, 128], bf16, tag="gA")
                v.tensor_scalar(gA[:], wbi[:, ksl], cic, None, AO.mult)
                v.scalar_tensor_tensor(Gr[:, ksl], wbr[:, ksl], crc, gA[:],
                                       AO.mult, AO.subtract)
                gB = gp.tile([128, 128], bf16, tag="gB")
                v.tensor_scalar(gB[:], wbr[:, ksl], cic, None, AO.mult)
                v.scalar_tensor_tensor(Gi[:, ksl], wbi[:, ksl], crc, gB[:],
                                       AO.mult, AO.add)
                v.tensor_scalar(Gn[:, ksl], Gi[:, ksl], -1.0, None, AO.mult)

            cf1r = cf1[:, 0:2304]
            cf1i = cf1[:, 2304:4608]
            cf1n = cf1[:, 4608:6912]
            w3r = cw3[:, 0:128]
            w3i = cw3[:, 128:256]
            w3n = cw3[:, 256:384]

            Abig = bp.tile([128, 18432], bf16, tag="Abig")
            Ubig = bp.tile([128, 18432], bf16, tag="Ubig")

            def ev_op(engine, dst, src):
                # pool/gpsimd cannot access PSUM on TRN2 hardware
                if engine == "act":
                    nc.scalar.activation(dst, src, AF.Copy)
                else:
                    nc.vector.tensor_copy(dst, src)

            # ---- Phase A: F1 + pivot-C (evict/DMA lagged one iteration) ----
            # DVE runs the G-build and Pool/ACT the crE expansion early; keep
            # phase-A evictions off DVE entirely (in-order queues would stall
            # F1 behind the G-build) and give Pool only the tail.
            EVA = ["act"] * 24
            # G pieces: (emit-after-iteration, k2, engine)
            GSCHED = {}
            _gk2 = 0
            for _it in range(2, 24):
                if _it % 4 != 1:
                    GSCHED[_it] = (_gk2, "dve")
                    _gk2 += 1
            # remaining k2 values flushed after phase A
            GREST = list(range(_gk2, N2))
            fa_pend = []  # (psum, rows, g, k2lo, nk)

            def fa_flush(idx):
                ab, rows, g, k2lo, nk = fa_pend[idx]
                stg = stp.tile([128, 1024], bf16, tag="stg")
                ev_op(EVA[idx], stg[:rows, :], ab[:rows, :])
                nc.sync.dma_start(
                    out=bass.AP(Abig.tensor,
                                Abig[:].offset + (16 * g) * 18432 + k2lo * 1024,
                                [[18432, 16], [1024, nk], [1, 1024]]),
                    in_=bass.AP(stg.tensor, stg[:].offset,
                                [[1024, 16 * nk], [1, 1024]]),
                )

            it = 0
            for si, (k2lo, k2hi) in enumerate(K2SPLIT):
                nk = k2hi - k2lo
                rows = nk * 16
                sbase = si * 128
                for g in range(8):
                    csl = slice(g * F1COLS + sbase, g * F1COLS + sbase + rows)
                    xr = xt[:, g * 1024 : g * 1024 + 512]
                    xi = xt[:, g * 1024 + 512 : (g + 1) * 1024]
                    ab = psa.tile([128, 1024], f32, tag="pX" if it % 2 == 0 else "pU")
                    nc.tensor.matmul(ab[:rows, 0:512], cf1r[:, csl], xr, start=True, stop=False)
                    nc.tensor.matmul(ab[:rows, 0:512], cf1n[:, csl], xi, start=False, stop=True)
                    nc.tensor.matmul(ab[:rows, 512:1024], cf1i[:, csl], xr, start=True, stop=False)
                    nc.tensor.matmul(ab[:rows, 512:1024], cf1r[:, csl], xi, start=False, stop=True)
                    fa_pend.append((ab, rows, g, k2lo, nk))
                    if it >= 1:
                        fa_flush(it - 1)
                    if it in GSCHED:
                        g_piece(*GSCHED[it])
                    it += 1
            fa_flush(it - 1)
            for _k2 in GREST:
                g_piece(_k2, "dve")
            # late const loads: needed only in phase C
            for v in range(3):
                vs = slice(v * 2176, (v + 1) * 2176)
                nc.sync.dma_start(out=ci2[:, vs], in_=d["ci2"][:, vs])

            # ---- Phase B: F3 + square + I1, software-pipelined by k2 ----
            # per-iteration emit: F3(k2), I1(k2-1), sq(k2), P2(k2), Zr(k2), ev(k2-1)
            XCE = ["act", "dve", "dve", "act", "dve", "act", "dve", "dve", "act",
                   "dve", "act", "dve", "dve", "act", "dve", "act", "dve", "act"]
            ZRE = ["pool", "pool", "dve", "pool", "pool", "dve", "pool", "pool",
                   "dve", "pool", "pool", "dve", "pool", "pool", "dve", "pool",
                   "pool", "dve"]
            EVB = ["act", "dve", "dve", "act", "act", "dve", "dve", "act", "dve",
                   "act", "dve", "dve", "act", "act", "dve", "dve", "act", "act"]
            zt = [None] * N2
            upst = [None] * N2

            def b_f3(k2):
                ar = Abig[:, k2 * 1024 : k2 * 1024 + 512]
                ai = Abig[:, k2 * 1024 + 512 : (k2 + 1) * 1024]
                xps = psa.tile([128, 1024], f32, tag="pX")
                nc.tensor.matmul(xps[:, 0:512], w3r, ar, start=True, stop=False)
                nc.tensor.matmul(xps[:, 0:512], w3n, ai, start=False, stop=True)
                nc.tensor.matmul(xps[:, 512:1024], w3i, ar, start=True, stop=False)
                nc.tensor.matmul(xps[:, 512:1024], w3r, ai, start=False, stop=True)
                return xps

            def b_i1(k2):
                z = zt[k2]
                gsl = slice(k2 * 128, (k2 + 1) * 128)
                ups = psa.tile([128, 1024], f32, tag="pU")
                nc.tensor.matmul(ups[:, 0:512], Gr[:, gsl], z[:, 0:512], start=True, stop=False)
                nc.tensor.matmul(ups[:, 0:512], Gn[:, gsl], z[:, 512:1024], start=False, stop=True)
                nc.tensor.matmul(ups[:, 512:1024], Gi[:, gsl], z[:, 0:512], start=True, stop=False)
                nc.tensor.matmul(ups[:, 512:1024], Gr[:, gsl], z[:, 512:1024], start=False, stop=True)
                upst[k2] = ups

            sqt = [None] * N2

            def b_sqp2(k2, xps):
                # hw: only one non-scalar input may be in PSUM; squares go
                # through ACT (single input), the cross product via an SBUF
                # copy of Xi
                sq = sp.tile([128, 1024], bf16, tag="sq")
                nc.scalar.activation(sq[:], xps[:], AF.Square)
                xic = xp2.tile([128, 512], bf16, tag="xic")
                if XCE[k2] == "act":
                    nc.scalar.activation(xic[:], xps[:, 512:1024], AF.Copy)
                else:
                    nc.vector.tensor_copy(xic[:], xps[:, 512:1024])
                sqt[k2] = sq
                z = zp.tile([128, 1024], bf16, tag="z")
                nc.vector.scalar_tensor_tensor(z[:, 512:1024], xps[:, 0:512], 2.0,
                                               xic[:], AO.mult, AO.mult)
                zt[k2] = z

            def b_zr(k2):
                v = nc.gpsimd if ZRE[k2] == "pool" else nc.vector
                v.tensor_tensor(zt[k2][:, 0:512], sqt[k2][:, 0:512], sqt[k2][:, 512:1024],
                                AO.subtract)

            def b_ev(k2):
                ev_op(EVB[k2], Ubig[:, k2 * 1024 : (k2 + 1) * 1024], upst[k2][:])

            for k2 in range(N2):
                xps = b_f3(k2)
                if k2 >= 3:
                    b_i1(k2 - 3)
                b_sqp2(k2, xps)
                if k2 >= 1:
                    b_zr(k2 - 1)
                if k2 >= 4:
                    b_ev(k2 - 4)
            b_zr(N2 - 1)
            for k2 in (N2 - 3, N2 - 2, N2 - 1):
                b_i1(k2)
            for k2 in (N2 - 4, N2 - 3, N2 - 2, N2 - 1):
                b_ev(k2)

            # ---- Phase C: pivot-D + I2 + |.| + store, pipelined by (g, blk) ----
            ci2r = ci2[:, 0:2176]
            ci2i = ci2[:, 2176:4352]
            ci2n = ci2[:, 4352:6528]
            POSTE = ["act", "act", "dve", "act", "dve", "dve"] * 4
            ADDE = ["pool", "pool", "dve"] * 8

            def c_pivd(g):
                tiles = []
                for bi, (j0, j1) in enumerate(JBLK):
                    cnt = j1 - j0
                    u2 = up.tile([128, 1024], bf16, tag=f"u2{bi}")
                    nc.sync.dma_start(
                        out=bass.AP(u2.tensor, u2[:].offset,
                                    [[1024, cnt * 18], [1, 1024]]),
                        in_=bass.AP(Ubig.tensor,
                                    Ubig[:].offset + (16 * g + j0) * 18432,
                                    [[18432, cnt], [1024, 18], [1, 1024]]),
                    )
                    tiles.append(u2)
                return tiles

            CB = [(0, 0, 119), (1, 119, 119), (2, 238, 34)]  # (blk, colbase, rows)

            def c_mm(g, bi, u2):
                j0, j1 = JBLK[bi]
                cnt = j1 - j0
                rows = CB[bi][2]
                parts = cnt * 18
                csl = slice(g * I2COLS + CB[bi][1], g * I2COLS + CB[bi][1] + rows)
                yps = psa.tile([128, 1024], f32, tag="pX" if (g + bi) % 2 == 0 else "pU")
                nc.tensor.matmul(yps[:rows, 0:512], ci2r[:parts, csl], u2[:parts, 0:512],
                                 start=True, stop=False)
                nc.tensor.matmul(yps[:rows, 0:512], ci2n[:parts, csl], u2[:parts, 512:1024],
                                 start=False, stop=True)
                nc.tensor.matmul(yps[:rows, 512:1024], ci2i[:parts, csl], u2[:parts, 0:512],
                                 start=True, stop=False)
                nc.tensor.matmul(yps[:rows, 512:1024], ci2r[:parts, csl], u2[:parts, 512:1024],
                                 start=False, stop=True)
                return yps

            def c_post(g, bi, yps, yy):
                rows = CB[bi][2]
                m = sp.tile([128, 1024], bf16, tag="m")
                pe = POSTE[(g * 3 + bi) % len(POSTE)]
                if pe == "act":
                    nc.scalar.activation(m[:rows, :], yps[:rows, :], AF.Square)
                else:
                    mc = xp2.tile([128, 1024], bf16, tag="mc")
                    nc.vector.tensor_copy(mc[:rows, :], yps[:rows, :])
                    nc.vector.tensor_mul(m[:rows, :], mc[:rows, :], mc[:rows, :])
                va = nc.gpsimd if ADDE[(g * 3 + bi) % len(ADDE)] == "pool" else nc.vector
                va.tensor_tensor(yy[:rows, bi * 512 : (bi + 1) * 512],
                                 m[:rows, 0:512], m[:rows, 512:1024], AO.add)

            # steady state per g: pivD(g+2), mm(g,*) with post lag 1 blk,
            # sqrt(g-1) right after post(g-1,2), out-DMA(g-2) last (so the
            # SP queue never parks on an unmet sqrt wait ahead of pivDs).
            # keep the PE p-state warm across the Ubig->pivot-D bubble with
            # throwaway matmuls (written, never read; psum tags rotate over them)
            for wf in range(10):
                warm = psa.tile([128, 1024], f32, tag="pX" if wf % 2 == 0 else "pU")
                nc.tensor.matmul(warm[:, 0:512], w3r, Abig[:, 0:512], start=True, stop=True)
                nc.tensor.matmul(warm[:, 512:1024], w3i, Abig[:, 0:512], start=True, stop=True)
            u2_0 = c_pivd(0)
            u2_1 = c_pivd(1)
            u2_t = {0: u2_0, 1: u2_1}
            yy_t = {}
            pend = []  # (g, bi, yps)
            fin = []   # g values whose posts are all emitted
            for g in range(8):
                yyg = yp.tile([128, 1536], bf16, tag="yy")
                nc.gpsimd.memset(yyg[0:YROWS, 1024:1536], 0.0)
                yy_t[g] = yyg
                if g + 2 < 8:
                    u2_t[g + 2] = c_pivd(g + 2)
                for bi in range(3):
                    yps = c_mm(g, bi, u2_t[g][bi])
                    if len(pend) >= 2:
                        pg, pbi, pyps = pend.pop(0)
                        c_post(pg, pbi, pyps, yy_t[pg])
                        if pbi == 2:
                            nc.scalar.activation(yy_t[pg][:YROWS, :], yy_t[pg][:YROWS, :], AF.Sqrt)
                            fin.append(pg)
                    pend.append((g, bi, yps))
                if len(fin) >= 2:
                    og = fin.pop(0)
                    nc.sync.dma_start(
                        out=yraw[0:YROWS, og * 1536 : (og + 1) * 1536],
                        in_=yy_t[og][:YROWS, :],
                    )
            while pend:
                pg, pbi, pyps = pend.pop(0)
                c_post(pg, pbi, pyps, yy_t[pg])
                if pbi == 2:
                    nc.scalar.activation(yy_t[pg][:YROWS, :], yy_t[pg][:YROWS, :], AF.Sqrt)
                    fin.append(pg)
            for og in fin:
                nc.sync.dma_start(
                    out=yraw[0:YROWS, og * 1536 : (og + 1) * 1536],
                    in_=yy_t[og][:YROWS, :],
                )

    nc.compile()
    return nc


_NC_CACHE = None


# ---------------- host-side orchestration ----------------
def _host_x(x_real, x_imag):
    """[Bc, 1024] f32 -> xt [128, 8192] bf16: p = 8j+n2, free = g*1024+plane*512+s."""
    out = np.empty((NCORES, 128, 8, 2, 512), BF)
    for cid in range(NCORES):
        rows = slice(cid * S, (cid + 1) * S)
        for pi, arr in enumerate((x_real, x_imag)):
            a = arr[rows].reshape(S, 8, 8, 16)          # (s, n2, g, j)
            a = a.transpose(3, 1, 2, 0)                 # (j, n2, g, s)
            out[cid, :, :, pi, :] = a.reshape(128, 8, S).astype(BF)
    return out.reshape(NCORES, 128, 8192)


def _build_wpack(w0r, w0i, wlr, wli):
    wp = np.zeros((128, 10), np.float32)
    wp[:, 0] = w0r[0:128]
    wp[:, 1] = w0i[0:128]
    wp[:, 2] = wlr[0:128]
    wp[:, 3] = wli[0:128]
    wp[:, 4] = wlr[128:256]
    wp[:, 5] = wli[128:256]
    wp[0, 6] = w0r[128]
    wp[0, 7] = w0i[128]
    wp[0, 8] = wlr[256]
    wp[0, 9] = wli[256]
    return wp


def _out_maps():
    """(rows, cols_in_yraw_per_g, out_col) for valid outputs."""
    rr, cc, oo = [], [], []
    for bi, (j0, j1) in enumerate(JBLK):
        for jp in range(j1 - j0):
            for qi in range(17):
                q = qi + 1
                r = jp * 17 + qi
                for g in range(8):
                    n = q * 128 + 16 * g + j0 + jp
                    if CROP0 <= n < CROP0 + CLASS_NUM:
                        rr.append(r)
                        cc.append(g * 1536 + bi * 512)
                        oo.append(n - CROP0)
    return np.array(rr), np.array(cc), np.array(oo)


_OUT_R, _OUT_C, _OUT_O = _out_maps()


def kernel(**inputs):
    global _NC_CACHE
    x_real = np.ascontiguousarray(inputs["x_real"], dtype=np.float32)
    x_imag = np.ascontiguousarray(inputs["x_imag"], dtype=np.float32)
    w0r = np.ascontiguousarray(inputs["w0_real"], dtype=np.float32)
    w0i = np.ascontiguousarray(inputs["w0_imag"], dtype=np.float32)
    wlr = np.ascontiguousarray(inputs["wl_real"], dtype=np.float32)
    wli = np.ascontiguousarray(inputs["wl_imag"], dtype=np.float32)

    xts = _host_x(x_real, x_imag)
    wp = _build_wpack(w0r, w0i, wlr, wli)

    const_maps = {nm: np.ascontiguousarray(arr) for nm, arr in CONSTS.items()}
    in_maps = []
    for cid in range(NCORES):
        m = {"xt": np.ascontiguousarray(xts[cid]), "wpack": wp}
        m.update(const_maps)
        in_maps.append(m)

    if _NC_CACHE is None:
        _NC_CACHE = build_nc()
    res = run_bass_kernel_spmd(_NC_CACHE, in_maps, core_ids=list(range(NCORES)))

    out = np.empty((B, CLASS_NUM), np.float32)
    for cid in range(NCORES):
        yr = np.asarray(res.results[cid]["yraw"], dtype=np.float32)  # [119, 12288]
        # gather: out[s, oo] = yr[rr, cc + s]
        sub = yr[_OUT_R[:, None], _OUT_C[:, None] + np.arange(S)[None, :]]  # [nv, S]
        out[cid * S : (cid + 1) * S, _OUT_O] = sub.T
    return out


# revision 62
# speedup vs baseline: 1.0568x; 1.0037x over previous
"""Trainium2 Bass kernel for nn_CNN_Comp_29240137351522 (dense_cnn), v2.

Math:  y = |IFFT_N( FFT_N(x)^2 * C )|,  C = FFT_N(w0)^2 * FFT_N(wl) / N
with N = 2304 (= 128*18).  2304 >= 2303 covers the autoconv h*h exactly, and
the final circular conv aliases y[n+2304] only onto n < 255, which the center
crop [255:2303) discards, so the cropped result is exact.

Device decomposition per core (data-parallel over batch, S = 512 samples):
  n = n2*128 + n1 (n2 in [0,18), x nonzero for n2 < 8),  k = 18*k1 + k2
  F1 (contract n2, block-diag over j = n1 mod 16, twiddle folded, bf16)
  pivot-C (DMA)   -> Abig[n1, (k2, plane, s)]
  F3 (contract n1, shared W128, bf16) -> X[k1, (k2, s)] in PSUM
  square (ACT dual-bank Square + DVE ops) -> Zr, P2 = 2*Xr*Xi (bf16)
  I1 (contract k1, G = C-row-scaled inverse DFT built on device, bf16)
  pivot-D (DMA)   -> u2[(j, k2), (plane, s)]
  I2 (contract k2, block-diag over j, bf16) + |.|^2 + sqrt -> yraw (bf16)
Host does data movement only: batch shard, x permutation into the F1-ready
layout, packing of weight vectors, and the output unscramble.
"""

import numpy as np
import ml_dtypes

import concourse.bass as bass
import concourse.bacc as bacc
import concourse.mybir as mybir
from concourse.tile import TileContext
from concourse.bass_utils import run_bass_kernel_spmd

# ---------------- static problem config ----------------
B, NX = 4096, 1024
K0, KL = 129, 257
N = 2304
N1, N2 = 128, 18
NCORES = 8
S = B // NCORES              # 512 samples per core, single chunk
CROP0 = 255
CLASS_NUM = 2048
K2SPLIT = ((0, 8), (8, 16), (16, 18))     # F1 column splits (k2-major)
JBLK = ((0, 7), (7, 14), (14, 16))        # I2 j-blocks per g
F1COLS = 288                               # 18*16 cols per g
I2COLS = 272                               # 16*17 cols per g
YROWS = 119                                # max I2 out rows (7*17)

f32 = mybir.dt.float32
f32r = mybir.dt.float32r
bf16 = mybir.dt.bfloat16
AO = mybir.AluOpType
AF = mybir.ActivationFunctionType

BF = ml_dtypes.bfloat16


def _w(num, den):
    return np.exp(-2j * np.pi * np.asarray(num, np.float64) / den)


# ---------------- host-side constant arrays ----------------
def _build_consts():
    c = {}
    n1g = np.arange(N1)
    k1g = np.arange(N1)
    k2g = np.arange(N2)

    # F1 lhsT [128, 8*288]: row p = 8j + n2 ; col g*288 + sbase + k2sub*16 + j
    # value W18^{n2 k2} * W2304^{(16g+j) k2}
    f1 = np.zeros((128, 8 * F1COLS), np.complex128)
    for g in range(8):
        for (k2lo, k2hi), sbase in zip(K2SPLIT, (0, 128, 256)):
            nk = k2hi - k2lo
            for k2 in range(k2lo, k2hi):
                for j in range(16):
                    n1 = 16 * g + j
                    col = g * F1COLS + sbase + j * nk + (k2 - k2lo)
                    vals = _w(np.arange(8) * k2, N2) * _w(n1 * k2, N)
                    f1[8 * j : 8 * j + 8, col] = vals
    c["cf1"] = np.concatenate(
        [f1.real, f1.imag, -f1.imag], axis=1).astype(BF)   # [128, 3*2304]

    # F3 lhsT (shared): W128[n1,k1], bf16 + f32 copy for the weight-DFT mms
    w3 = _w(np.outer(n1g, k1g), N1)
    w3cat = np.concatenate([w3.real, w3.imag, -w3.imag], axis=1)
    c["cw3"] = w3cat.astype(BF)                            # [128, 384]
    c["cw3f"] = w3cat.astype(np.float32)                   # [128, 384]

    # inverse-DFT base tiled over k2, divided by N (folds the 1/N of C):
    # cwiB[:, v*2304 + k2*128 + p] = {Re,Im}(W128^{-k1 p}) / N
    wi = _w(-np.outer(k1g, n1g), N1) / N
    blk = np.concatenate([np.tile(wi.real, (1, N2)), np.tile(wi.imag, (1, N2))], axis=1)
    c["cwiB"] = blk.astype(BF)                             # [128, 2*2304]

    # I2 lhsT [128, 8*272]: per (g, jb): rows p = j'*18 + k2, col g*272 + base
    # + j'*17 + (q-1); value W18^{-q k2} * W2304^{-(16g+j0+j') k2}, q in [1,18)
    i2 = np.zeros((128, 8 * I2COLS), np.complex128)
    qg = np.arange(1, 18)
    for g in range(8):
        base = 0
        for (j0, j1) in JBLK:
            for jp in range(j1 - j0):
                n1 = 16 * g + j0 + jp
                blkv = _w(-np.outer(k2g, qg), N2) * _w(-n1 * k2g, N)[:, None]
                rows = slice(jp * 18, jp * 18 + 18)
                cols = slice(g * I2COLS + base + jp * 17, g * I2COLS + base + (jp + 1) * 17)
                i2[rows, cols] = blkv
            base += (j1 - j0) * 17
    c["ci2"] = np.concatenate(
        [i2.real, i2.imag, -i2.imag], axis=1).astype(BF)   # [128, 3*2176]

    # weight-DFT rhs constants (f32), packed into one [128, 272] tensor:
    # cols 0:18 ct1r | 18:36 ct1i | 36:54 ct2r | 54:72 ct2i
    # row0 cols 72:90 te1r | 90:108 te1i | 108:126 te2r | 126:144 te2i
    # row0 cols 144:272 ones (128)
    nh = np.arange(128)
    sm = np.zeros((128, 272), np.float32)
    t1 = _w(np.outer(nh, k2g), N)
    sm[:, 0:18] = t1.real
    sm[:, 18:36] = t1.imag
    t2 = _w(np.outer(nh, k2g), N) * _w(k2g, N2)[None, :]
    sm[:, 36:54] = t2.real
    sm[:, 54:72] = t2.imag
    te1 = _w(k2g, N2)
    sm[0, 72:90] = te1.real
    sm[0, 90:108] = te1.imag
    te2 = _w(k2g, 9)
    sm[0, 108:126] = te2.real
    sm[0, 126:144] = te2.imag
    sm[0, 144:272] = 1.0
    c["csm"] = sm

    return c


CONSTS = _build_consts()


# ---------------- bass kernel builder ----------------
def build_nc():
    nc = bacc.Bacc("TRN2", target_bir_lowering=False, debug=False, num_devices=NCORES)

    d = {}
    d["xt"] = nc.dram_tensor("xt", [128, 2 * 4096], bf16, kind="ExternalInput")
    d["wpack"] = nc.dram_tensor("wpack", [128, 10], f32, kind="ExternalInput")
    cdt = {"cw3f": f32, "csm": f32}
    for nm, arr in CONSTS.items():
        d[nm] = nc.dram_tensor(nm, list(arr.shape), cdt.get(nm, bf16), kind="ExternalInput")
    yraw = nc.dram_tensor("yraw", [YROWS, 8 * 1536], bf16, kind="ExternalOutput")

    with TileContext(nc) as tc:
        with (
            tc.tile_pool(name="cp", bufs=1) as cp,          # persistent consts
            tc.tile_pool(name="bp", bufs=1) as bp,          # Abig / Ubig / G
            tc.tile_pool(name="sp", bufs=3) as sp,          # rotating stage tiles
            tc.tile_pool(name="gp", bufs=2) as gp,          # G-build temporaries
            tc.tile_pool(name="stp", bufs=4) as stp,        # pivot-C staging
            tc.tile_pool(name="xp2", bufs=2) as xp2,        # xi/y copies
            tc.tile_pool(name="up", bufs=3) as up,          # u2 tiles
            tc.tile_pool(name="yp", bufs=4) as yp,          # yy tiles
            tc.tile_pool(name="zp", bufs=4) as zp,          # z tiles
            tc.tile_pool(name="tp", bufs=1) as tp,          # small f32 tmps
            tc.tile_pool(name="psa", bufs=2, space="PSUM") as psa,
        ):
            # ---- const + input DMAs (sync engine; ordered by need) ----
            # xt free layout: g*1024 + plane*512 + s  (per-g slices contiguous)
            wpk = cp.tile([128, 10], f32, tag="wpack")
            nc.sync.dma_start(out=wpk[:], in_=d["wpack"][:, :])
            csm = cp.tile([128, 272], f32, tag="csm")
            nc.sync.dma_start(out=csm[:], in_=d["csm"][:, :])
            cw3f = cp.tile([128, 384], f32, tag="cw3f")
            nc.sync.dma_start(out=cw3f[:], in_=d["cw3f"][:, :])
            cf1 = cp.tile([128, 3 * 2304], bf16, tag="cf1")
            for v in (0, 2, 1):
                vs = slice(v * 2304, (v + 1) * 2304)
                nc.sync.dma_start(out=cf1[:, vs], in_=d["cf1"][:, vs])
            xt = cp.tile([128, 8192], bf16, tag="xt")
            for g in range(8):
                gs = slice(g * 1024, (g + 1) * 1024)
                nc.sync.dma_start(out=xt[:, gs], in_=d["xt"][:, gs])
            cw3 = cp.tile([128, 384], bf16, tag="cw3")
            nc.sync.dma_start(out=cw3[:], in_=d["cw3"][:, :])
            cwiB = cp.tile([128, 2 * 2304], bf16, tag="cwiB")
            nc.sync.dma_start(out=cwiB[:], in_=d["cwiB"][:, :])
            ci2 = cp.tile([128, 3 * 2176], bf16, tag="ci2")

            # ---- weight DFT -> C (without 1/N; folded into cwiB) ----
            # rhs builds [rows, 18] f32  (complex products via DVE small ops)
            def cplx_rhs(rows, tr, ti, cr, ci, outr, outi):
                # (cr + i ci) * (tr + i ti); cr/ci are [rows,1] scalar APs
                t = tp.tile([128, 18], f32, tag="wtmp")
                nc.vector.tensor_scalar(t[:rows, :], ti, ci, None, AO.mult)
                nc.vector.scalar_tensor_tensor(outr, tr, cr, t[:rows, :], AO.mult, AO.subtract)
                t2 = tp.tile([128, 18], f32, tag="wtmp2")
                nc.vector.tensor_scalar(t2[:rows, :], tr, ci, None, AO.mult)
                nc.vector.scalar_tensor_tensor(outi, ti, cr, t2[:rows, :], AO.mult, AO.add)

            rhs0 = tp.tile([128, 36], f32, tag="rhs0")
            cplx_rhs(128, csm[:, 0:18], csm[:, 18:36], wpk[:, 0:1], wpk[:, 1:2],
                     rhs0[:, 0:18], rhs0[:, 18:36])
            tl0 = tp.tile([1, 36], f32, tag="tl0")
            cplx_rhs(1, csm[0:1, 72:90], csm[0:1, 90:108], wpk[0:1, 6:7], wpk[0:1, 7:8],
                     tl0[:, 0:18], tl0[:, 18:36])
            rhs1 = tp.tile([128, 36], f32, tag="rhs1")
            cplx_rhs(128, csm[:, 0:18], csm[:, 18:36], wpk[:, 2:3], wpk[:, 3:4],
                     rhs1[:, 0:18], rhs1[:, 18:36])
            rhs2 = tp.tile([128, 36], f32, tag="rhs2")
            cplx_rhs(128, csm[:, 36:54], csm[:, 54:72], wpk[:, 4:5], wpk[:, 5:6],
                     rhs2[:, 0:18], rhs2[:, 18:36])
            tl2 = tp.tile([1, 36], f32, tag="tl2")
            cplx_rhs(1, csm[0:1, 108:126], csm[0:1, 126:144], wpk[0:1, 8:9], wpk[0:1, 9:10],
                     tl2[:, 0:18], tl2[:, 18:36])

            w3fr = cw3f[:, 0:128]
            w3fi = cw3f[:, 128:256]
            w3fn = cw3f[:, 256:384]
            onesf = csm[0:1, 144:272]

            w0ps = psa.tile([128, 36], f32, tag="pX")
            nc.tensor.matmul(w0ps[:, 0:18], w3fr, rhs0[:, 0:18], start=True, stop=False)
            nc.tensor.matmul(w0ps[:, 0:18], w3fn, rhs0[:, 18:36], start=False, stop=False)
            nc.tensor.matmul(w0ps[:, 0:18], onesf, tl0[:, 0:18], start=False, stop=True)
            nc.tensor.matmul(w0ps[:, 18:36], w3fi, rhs0[:, 0:18], start=True, stop=False)
            nc.tensor.matmul(w0ps[:, 18:36], w3fr, rhs0[:, 18:36], start=False, stop=False)
            nc.tensor.matmul(w0ps[:, 18:36], onesf, tl0[:, 18:36], start=False, stop=True)
            wlps = psa.tile([128, 36], f32, tag="pU")
            nc.tensor.matmul(wlps[:, 0:18], w3fr, rhs1[:, 0:18], start=True, stop=False)
            nc.tensor.matmul(wlps[:, 0:18], w3fn, rhs1[:, 18:36], start=False, stop=False)
            nc.tensor.matmul(wlps[:, 0:18], w3fr, rhs2[:, 0:18], start=False, stop=False)
            nc.tensor.matmul(wlps[:, 0:18], w3fn, rhs2[:, 18:36], start=False, stop=False)
            nc.tensor.matmul(wlps[:, 0:18], onesf, tl2[:, 0:18], start=False, stop=True)
            nc.tensor.matmul(wlps[:, 18:36], w3fi, rhs1[:, 0:18], start=True, stop=False)
            nc.tensor.matmul(wlps[:, 18:36], w3fr, rhs1[:, 18:36], start=False, stop=False)
            nc.tensor.matmul(wlps[:, 18:36], w3fi, rhs2[:, 0:18], start=False, stop=False)
            nc.tensor.matmul(wlps[:, 18:36], w3fr, rhs2[:, 18:36], start=False, stop=False)
            nc.tensor.matmul(wlps[:, 18:36], onesf, tl2[:, 18:36], start=False, stop=True)

            # bridge the W-DFT -> F1 PE gap so F1 starts at full p-state
            for wf in range(6):
                warm0 = psa.tile([128, 1024], f32, tag="pX" if wf % 2 == 0 else "pU")
                nc.tensor.matmul(warm0[:, 0:272], cw3f[:, 0:128], csm[:, 0:272],
                                 start=True, stop=True)
            w0sb = tp.tile([128, 36], f32, tag="w0sb")
            nc.scalar.activation(w0sb[:], w0ps[:], AF.Copy)
            wlsb = tp.tile([128, 36], f32, tag="wlsb")
            nc.scalar.activation(wlsb[:], wlps[:], AF.Copy)

            # C*N = W0^2 * WL  (f32, [128, 18] each)
            ca = tp.tile([128, 18], f32, tag="ca")
            cb = tp.tile([128, 18], f32, tag="cb")
            cm1 = tp.tile([128, 18], f32, tag="cm1")
            cm2 = tp.tile([128, 18], f32, tag="cm2")
            nc.vector.tensor_mul(cm1[:], w0sb[:, 0:18], w0sb[:, 0:18])
            nc.vector.tensor_mul(cm2[:], w0sb[:, 18:36], w0sb[:, 18:36])
            nc.vector.tensor_sub(ca[:], cm1[:], cm2[:])
            nc.vector.scalar_tensor_tensor(cb[:], w0sb[:, 0:18], 2.0, w0sb[:, 18:36],
                                           AO.mult, AO.mult)
            crn = tp.tile([128, 18], f32, tag="crn")
            cin = tp.tile([128, 18], f32, tag="cin")
            nc.vector.tensor_mul(cm1[:], ca[:], wlsb[:, 0:18])
            nc.vector.tensor_mul(cm2[:], cb[:], wlsb[:, 18:36])
            nc.vector.tensor_sub(crn[:], cm1[:], cm2[:])
            nc.vector.tensor_mul(cm1[:], ca[:], wlsb[:, 18:36])
            nc.vector.tensor_mul(cm2[:], cb[:], wlsb[:, 0:18])
            nc.vector.tensor_add(cin[:], cm1[:], cm2[:])

            # ---- G build: G = (wi/N) * C*N, [128, 2304] bf16 x3 ----
            # per-k2 pieces on DVE or Pool (SBUF-only, hw-legal); emitted
            # interleaved into phase A so they don't block phase-A evictions
            Gr = bp.tile([128, 2304], bf16, tag="Gr")
            Gi = bp.tile([128, 2304], bf16, tag="Gi")
            Gn = bp.tile([128, 2304], bf16, tag="Gn")
            wbr = cwiB[:, 0:2304]
            wbi = cwiB[:, 2304:4608]

            def g_piece(k2, eng):
                v = nc.vector if eng == "dve" else nc.gpsimd
                ksl = slice(k2 * 128, (k2 + 1) * 128)
                crc = crn[:, k2:k2 + 1]
                cic = cin[:, k2:k2 + 1]
                gA = gp.tile([128, 128], bf16, tag="gA")
                v.tensor_scalar(gA[:], wbi[:, ksl], cic, None, AO.mult)
                v.scalar_tensor_tensor(Gr[:, ksl], wbr[:, ksl], crc, gA[:],
                                       AO.mult, AO.subtract)
                gB = gp.tile([128, 128], bf16, tag="gB")
                v.tensor_scalar(gB[:], wbr[:, ksl], cic, None, AO.mult)
                v.scalar_tensor_tensor(Gi[:, ksl], wbi[:, ksl], crc, gB[:],
                                       AO.mult, AO.add)
                v.tensor_scalar(Gn[:, ksl], Gi[:, ksl], -1.0, None, AO.mult)

            cf1r = cf1[:, 0:2304]
            cf1i = cf1[:, 2304:4608]
            cf1n = cf1[:, 4608:6912]
            w3r = cw3[:, 0:128]
            w3i = cw3[:, 128:256]
            w3n = cw3[:, 256:384]

            Abig = bp.tile([128, 18432], bf16, tag="Abig")
            Ubig = bp.tile([128, 18432], bf16, tag="Ubig")

            def ev_op(engine, dst, src):
                # pool/gpsimd cannot access PSUM on TRN2 hardware
                if engine == "act":
                    nc.scalar.activation(dst, src, AF.Copy)
                else:
                    nc.vector.tensor_copy(dst, src)

            # ---- Phase A: F1 + pivot-C (evict/DMA lagged one iteration) ----
            # DVE runs the G-build and Pool/ACT the crE expansion early; keep
            # phase-A evictions off DVE entirely (in-order queues would stall
            # F1 behind the G-build) and give Pool only the tail.
            EVA = ["act"] * 24
            # G pieces: (emit-after-iteration, k2, engine)
            GSCHED = {}
            _gk2 = 0
            for _it in range(2, 24):
                if _it % 4 != 1:
                    GSCHED[_it] = (_gk2, "dve")
                    _gk2 += 1
            # remaining k2 values flushed after phase A
            GREST = list(range(_gk2, N2))
            fa_pend = []  # (psum, rows, g, k2lo, nk)

            def fa_flush(idx):
                ab, rows, g, k2lo, nk = fa_pend[idx]
                stg = stp.tile([128, 1024], bf16, tag="stg")
                ev_op(EVA[idx], stg[:rows, :], ab[:rows, :])
                nc.sync.dma_start(
                    out=bass.AP(Abig.tensor,
                                Abig[:].offset + (16 * g) * 18432 + k2lo * 1024,
                                [[18432, 16], [1024, nk], [1, 1024]]),
                    in_=bass.AP(stg.tensor, stg[:].offset,
                                [[1024, 16 * nk], [1, 1024]]),
                )

            it = 0
            for si, (k2lo, k2hi) in enumerate(K2SPLIT):
                nk = k2hi - k2lo
                rows = nk * 16
                sbase = si * 128
                for g in range(8):
                    csl = slice(g * F1COLS + sbase, g * F1COLS + sbase + rows)
                    xr = xt[:, g * 1024 : g * 1024 + 512]
                    xi = xt[:, g * 1024 + 512 : (g + 1) * 1024]
                    ab = psa.tile([128, 1024], f32, tag="pX" if it % 2 == 0 else "pU")
                    nc.tensor.matmul(ab[:rows, 0:512], cf1r[:, csl], xr, start=True, stop=False)
                    nc.tensor.matmul(ab[:rows, 0:512], cf1n[:, csl], xi, start=False, stop=True)
                    nc.tensor.matmul(ab[:rows, 512:1024], cf1i[:, csl], xr, start=True, stop=False)
                    nc.tensor.matmul(ab[:rows, 512:1024], cf1r[:, csl], xi, start=False, stop=True)
                    fa_pend.append((ab, rows, g, k2lo, nk))
                    if it >= 1:
                        fa_flush(it - 1)
                    if it in GSCHED:
                        g_piece(*GSCHED[it])
                    it += 1
            fa_flush(it - 1)
            for _k2 in GREST:
                g_piece(_k2, "dve")
            # late const loads: needed only in phase C
            for v in range(3):
                vs = slice(v * 2176, (v + 1) * 2176)
                nc.sync.dma_start(out=ci2[:, vs], in_=d["ci2"][:, vs])

            # ---- Phase B: F3 + square + I1, software-pipelined by k2 ----
            # per-iteration emit: F3(k2), I1(k2-1), sq(k2), P2(k2), Zr(k2), ev(k2-1)
            XCE = ["act", "dve", "dve", "act", "dve", "act", "dve", "dve", "act",
                   "dve", "act", "dve", "dve", "act", "dve", "act", "dve", "act"]
            ZRE = ["pool", "pool", "dve", "pool", "pool", "dve", "pool", "pool",
                   "dve", "pool", "pool", "dve", "pool", "pool", "dve", "pool",
                   "pool", "dve"]
            EVB = ["act", "dve", "dve", "act", "act", "dve", "dve", "act", "dve",
                   "act", "dve", "dve", "act", "act", "dve", "dve", "act", "act"]
            zt = [None] * N2
            upst = [None] * N2

            def b_f3(k2):
                ar = Abig[:, k2 * 1024 : k2 * 1024 + 512]
                ai = Abig[:, k2 * 1024 + 512 : (k2 + 1) * 1024]
                xps = psa.tile([128, 1024], f32, tag="pX")
                nc.tensor.matmul(xps[:, 0:512], w3r, ar, start=True, stop=False)
                nc.tensor.matmul(xps[:, 0:512], w3n, ai, start=False, stop=True)
                nc.tensor.matmul(xps[:, 512:1024], w3i, ar, start=True, stop=False)
                nc.tensor.matmul(xps[:, 512:1024], w3r, ai, start=False, stop=True)
                return xps

            def b_i1(k2):
                z = zt[k2]
                gsl = slice(k2 * 128, (k2 + 1) * 128)
                ups = psa.tile([128, 1024], f32, tag="pU")
                nc.tensor.matmul(ups[:, 0:512], Gr[:, gsl], z[:, 0:512], start=True, stop=False)
                nc.tensor.matmul(ups[:, 0:512], Gn[:, gsl], z[:, 512:1024], start=False, stop=True)
                nc.tensor.matmul(ups[:, 512:1024], Gi[:, gsl], z[:, 0:512], start=True, stop=False)
                nc.tensor.matmul(ups[:, 512:1024], Gr[:, gsl], z[:, 512:1024], start=False, stop=True)
                upst[k2] = ups

            sqt = [None] * N2

            def b_sqp2(k2, xps):
                # hw: only one non-scalar input may be in PSUM; squares go
                # through ACT (single input), the cross product via an SBUF
                # copy of Xi
                sq = sp.tile([128, 1024], bf16, tag="sq")
                nc.scalar.activation(sq[:], xps[:], AF.Square)
                xic = xp2.tile([128, 512], bf16, tag="xic")
                if XCE[k2] == "act":
                    nc.scalar.activation(xic[:], xps[:, 512:1024], AF.Copy)
                else:
                    nc.vector.tensor_copy(xic[:], xps[:, 512:1024])
                sqt[k2] = sq
                z = zp.tile([128, 1024], bf16, tag="z")
                nc.vector.scalar_tensor_tensor(z[:, 512:1024], xps[:, 0:512], 2.0,
                                               xic[:], AO.mult, AO.mult)
                zt[k2] = z

            def b_zr(k2):
                v = nc.gpsimd if ZRE[k2] == "pool" else nc.vector
                v.tensor_tensor(zt[k2][:, 0:512], sqt[k2][:, 0:512], sqt[k2][:, 512:1024],
                                AO.subtract)

            def b_ev(k2):
                ev_op(EVB[k2], Ubig[:, k2 * 1024 : (k2 + 1) * 1024], upst[k2][:])

            for k2 in range(N2):
                xps = b_f3(k2)
                if k2 >= 3:
                    b_i1(k2 - 3)
                b_sqp2(k2, xps)
                if k2 >= 1:
                    b_zr(k2 - 1)
                if k2 >= 4:
                    b_ev(k2 - 4)
            b_zr(N2 - 1)
            for k2 in (N2 - 3, N2 - 2, N2 - 1):
                b_i1(k2)
            for k2 in (N2 - 4, N2 - 3, N2 - 2, N2 - 1):
                b_ev(k2)

            # ---- Phase C: pivot-D + I2 + |.| + store, pipelined by (g, blk) ----
            ci2r = ci2[:, 0:2176]
            ci2i = ci2[:, 2176:4352]
            ci2n = ci2[:, 4352:6528]
            POSTE = ["act", "act", "dve", "act", "dve", "dve"] * 4
            ADDE = ["pool", "pool", "dve"] * 8

            def c_pivd(g):
                tiles = []
                for bi, (j0, j1) in enumerate(JBLK):
                    cnt = j1 - j0
                    u2 = up.tile([128, 1024], bf16, tag=f"u2{bi}")
                    nc.sync.dma_start(
                        out=bass.AP(u2.tensor, u2[:].offset,
                                    [[1024, cnt * 18], [1, 1024]]),
                        in_=bass.AP(Ubig.tensor,
                                    Ubig[:].offset + (16 * g + j0) * 18432,
                                    [[18432, cnt], [1024, 18], [1, 1024]]),
                    )
                    tiles.append(u2)
                return tiles

            CB = [(0, 0, 119), (1, 119, 119), (2, 238, 34)]  # (blk, colbase, rows)

            def c_mm(g, bi, u2):
                j0, j1 = JBLK[bi]
                cnt = j1 - j0
                rows = CB[bi][2]
                parts = cnt * 18
                csl = slice(g * I2COLS + CB[bi][1], g * I2COLS + CB[bi][1] + rows)
                yps = psa.tile([128, 1024], f32, tag="pX" if (g + bi) % 2 == 0 else "pU")
                nc.tensor.matmul(yps[:rows, 0:512], ci2r[:parts, csl], u2[:parts, 0:512],
                                 start=True, stop=False)
                nc.tensor.matmul(yps[:rows, 0:512], ci2n[:parts, csl], u2[:parts, 512:1024],
                                 start=False, stop=True)
                nc.tensor.matmul(yps[:rows, 512:1024], ci2i[:parts, csl], u2[:parts, 0:512],
                                 start=True, stop=False)
                nc.tensor.matmul(yps[:rows, 512:1024], ci2r[:parts, csl], u2[:parts, 512:1024],
                                 start=False, stop=True)
                return yps

            def c_post(g, bi, yps, yy):
                rows = CB[bi][2]
                m = sp.tile([128, 1024], bf16, tag="m")
                pe = POSTE[(g * 3 + bi) % len(POSTE)]
                if pe == "act":
                    nc.scalar.activation(m[:rows, :], yps[:rows, :], AF.Square)
                else:
                    mc = xp2.tile([128, 1024], bf16, tag="mc")
                    nc.vector.tensor_copy(mc[:rows, :], yps[:rows, :])
                    nc.vector.tensor_mul(m[:rows, :], mc[:rows, :], mc[:rows, :])
                va = nc.gpsimd if ADDE[(g * 3 + bi) % len(ADDE)] == "pool" else nc.vector
                va.tensor_tensor(yy[:rows, bi * 512 : (bi + 1) * 512],
                                 m[:rows, 0:512], m[:rows, 512:1024], AO.add)

            # steady state per g: pivD(g+2), mm(g,*) with post lag 1 blk,
            # sqrt(g-1) right after post(g-1,2), out-DMA(g-2) last (so the
            # SP queue never parks on an unmet sqrt wait ahead of pivDs).
            # keep the PE p-state warm across the Ubig->pivot-D bubble with
            # throwaway matmuls (written, never read; psum tags rotate over them)
            for wf in range(10):
                warm = psa.tile([128, 1024], f32, tag="pX" if wf % 2 == 0 else "pU")
                nc.tensor.matmul(warm[:, 0:512], w3r, Abig[:, 0:512], start=True, stop=True)
                nc.tensor.matmul(warm[:, 512:1024], w3i, Abig[:, 0:512], start=True, stop=True)
            u2_0 = c_pivd(0)
            u2_1 = c_pivd(1)
            u2_t = {0: u2_0, 1: u2_1}
            yy_t = {}
            pend = []  # (g, bi, yps)
            fin = []   # g values whose posts are all emitted
            for g in range(8):
                yyg = yp.tile([128, 1536], bf16, tag="yy")
                nc.gpsimd.memset(yyg[0:YROWS, 1024:1536], 0.0)
                yy_t[g] = yyg
                if g + 2 < 8:
                    u2_t[g + 2] = c_pivd(g + 2)
                for bi in range(3):
                    yps = c_mm(g, bi, u2_t[g][bi])
                    if len(pend) >= 2:
                        pg, pbi, pyps = pend.pop(0)
                        c_post(pg, pbi, pyps, yy_t[pg])
                        if pbi == 2:
                            nc.scalar.activation(yy_t[pg][:YROWS, :], yy_t[pg][:YROWS, :], AF.Sqrt)
                            fin.append(pg)
                    pend.append((g, bi, yps))
                if len(fin) >= 2:
                    og = fin.pop(0)
                    nc.sync.dma_start(
                        out=yraw[0:YROWS, og * 1536 : (og + 1) * 1536],
                        in_=yy_t[og][:YROWS, :],
                    )
            while pend:
                pg, pbi, pyps = pend.pop(0)
                c_post(pg, pbi, pyps, yy_t[pg])
                if pbi == 2:
                    nc.scalar.activation(yy_t[pg][:YROWS, :], yy_t[pg][:YROWS, :], AF.Sqrt)
                    fin.append(pg)
            for og in fin:
                nc.sync.dma_start(
                    out=yraw[0:YROWS, og * 1536 : (og + 1) * 1536],
                    in_=yy_t[og][:YROWS, :],
                )

    nc.compile()
    return nc


_NC_CACHE = None


# ---------------- host-side orchestration ----------------
def _host_x(x_real, x_imag):
    """[Bc, 1024] f32 -> xt [128, 8192] bf16: p = 8j+n2, free = g*1024+plane*512+s."""
    out = np.empty((NCORES, 128, 8, 2, 512), BF)
    for cid in range(NCORES):
        rows = slice(cid * S, (cid + 1) * S)
        for pi, arr in enumerate((x_real, x_imag)):
            a = arr[rows].reshape(S, 8, 8, 16)          # (s, n2, g, j)
            a = a.transpose(3, 1, 2, 0)                 # (j, n2, g, s)
            out[cid, :, :, pi, :] = a.reshape(128, 8, S).astype(BF)
    return out.reshape(NCORES, 128, 8192)


def _build_wpack(w0r, w0i, wlr, wli):
    wp = np.zeros((128, 10), np.float32)
    wp[:, 0] = w0r[0:128]
    wp[:, 1] = w0i[0:128]
    wp[:, 2] = wlr[0:128]
    wp[:, 3] = wli[0:128]
    wp[:, 4] = wlr[128:256]
    wp[:, 5] = wli[128:256]
    wp[0, 6] = w0r[128]
    wp[0, 7] = w0i[128]
    wp[0, 8] = wlr[256]
    wp[0, 9] = wli[256]
    return wp


def _out_maps():
    """(rows, cols_in_yraw_per_g, out_col) for valid outputs."""
    rr, cc, oo = [], [], []
    for bi, (j0, j1) in enumerate(JBLK):
        for jp in range(j1 - j0):
            for qi in range(17):
                q = qi + 1
                r = jp * 17 + qi
                for g in range(8):
                    n = q * 128 + 16 * g + j0 + jp
                    if CROP0 <= n < CROP0 + CLASS_NUM:
                        rr.append(r)
                        cc.append(g * 1536 + bi * 512)
                        oo.append(n - CROP0)
    return np.array(rr), np.array(cc), np.array(oo)


_OUT_R, _OUT_C, _OUT_O = _out_maps()


def kernel(**inputs):
    global _NC_CACHE
    x_real = np.ascontiguousarray(inputs["x_real"], dtype=np.float32)
    x_imag = np.ascontiguousarray(inputs["x_imag"], dtype=np.float32)
    w0r = np.ascontiguousarray(inputs["w0_real"], dtype=np.float32)
    w0i = np.ascontiguousarray(inputs["w0_imag"], dtype=np.float32)
    wlr = np.ascontiguousarray(inputs["wl_real"], dtype=np.float32)
    wli = np.ascontiguousarray(inputs["wl_imag"], dtype=np.float32)

    xts = _host_x(x_real, x_imag)
    wp = _build_wpack(w0r, w0i, wlr, wli)

    const_maps = {nm: np.ascontiguousarray(arr) for nm, arr in CONSTS.items()}
    in_maps = []
    for cid in range(NCORES):
        m = {"xt": np.ascontiguousarray(xts[cid]), "wpack": wp}
        m.update(const_maps)
        in_maps.append(m)

    if _NC_CACHE is None:
        _NC_CACHE = build_nc()
    res = run_bass_kernel_spmd(_NC_CACHE, in_maps, core_ids=list(range(NCORES)))

    out = np.empty((B, CLASS_NUM), np.float32)
    for cid in range(NCORES):
        yr = np.asarray(res.results[cid]["yraw"], dtype=np.float32)  # [119, 12288]
        # gather: out[s, oo] = yr[rr, cc + s]
        sub = yr[_OUT_R[:, None], _OUT_C[:, None] + np.arange(S)[None, :]]  # [nv, S]
        out[cid * S : (cid + 1) * S, _OUT_O] = sub.T
    return out
